# revision 11
# baseline (speedup 1.0000x reference)
"""Causal multi-head attention on 8 TRN2 NeuronCores.

Problem: B=4, H=16, S=2048, D=128 fp32 causal attention.
Sharding: batch*heads (64) split 8-per-core across the 8 cores; each core
computes its heads fully independently (no collectives).

Per-core kernel strategy (f32 accumulation):
  - scores computed TRANSPOSED: S^T[k,q] = K_j @ Q^T per (k-block j of 128,
    q-group g of 512), causal blocks only, into 2-bank PSUM chunks (<=1024)
    with a 3-deep chunk pipeline
  - ALL non-diagonal score blocks of groups g2/g3 run as fp8e4 DoubleRow
    matmuls at 2x PE rate (cost-model 0.5 cycles/row): the stationary K side
    carries hi + lo/16 double-fp8 (~8 mantissa bits), the moving Q side is
    one-sided e4m3 with a /16 second slot. Host pre-quantizes Q/K; measured
    output rel err ~1.0% vs the 2e-2 gate. fp8 on g0/g1/diagonal blocks is
    deliberately avoided: early rows have few keys and large output norms,
    so they dominate the error weighting.
  - exp mostly on ScalarE (PSUM -> SBUF bf16, 1/sqrt(D) folded into the
    activation scale); four late chunks per head offloaded to VectorE via an
    exp2 bit-trick (one tensor_scalar: bf16 bits = rint(x*log2e*128 +
    127*128 - 7.5), f32->int16 convert aliased over the bf16 tile)
  - diagonal 128x128 blocks masked with a constant triangular tile on DVE
  - PV: out[q,:] = P^T_slice.T @ [V_j | ones]; the appended ones column
    yields the softmax denominator in the same accumulation. Output lands
    directly in [q, d] layout.
  - normalize with VectorE reciprocal + per-partition tensor_scalar mult
  - packed single-DMA startup tile (mask consts + first K/Q block); per-head
    unit order g0,g3,g1,g2 balances exp-column supply against PV drains;
    per-engine pacing clocks keep the PE fed with PV filler work
"""

import sys

import numpy as np
import ml_dtypes

for _p in ("/opt/trn_rl_repo", "/root/.axon_site/_ro/trn_rl_repo"):
    try:
        import concourse  # noqa: F401
        break
    except ImportError:
        if _p not in sys.path:
            sys.path.append(_p)

B, H, S, D = 4, 16, 2048, 128
N_CORES = 8
HPC = (B * H) // N_CORES  # heads per core = 8
QB = 128                  # q/k block
GW = 512                  # q group width
NG = S // GW              # 4 groups per head
NJ = S // QB              # 16 k blocks
VW = D + 1                # V augmented with ones column = 129
SCALE = 1.0 / float(np.sqrt(D))

_BF16 = ml_dtypes.bfloat16

_CACHE = {}


def _build():
    import concourse.bass as bass  # noqa: F401
    import concourse.mybir as mybir
    from concourse import bacc
    from concourse.tile import TileContext

    f32 = mybir.dt.float32
    bf16 = mybir.dt.bfloat16
    i16 = mybir.dt.int16
    EXP = mybir.ActivationFunctionType.Exp
    # chunks routed to the bit-trick exp on VectorE / GpSimd: (g, chunk_idx).
    # Late chunks only — their PV consumption comes last in each q-chain, so
    # the slower engines' latency hides behind ScalarE's pipeline.
    import os as _os
    _dve = _os.environ.get("DVE_SET",
                           "(3,0);(3,1);(3,2);(3,3);(3,4);(3,5)")
    _gps = _os.environ.get("GPS_SET", "")

    def _parse(s):
        out = set()
        for part in s.split(";"):
            part = part.strip().strip("()")
            if part:
                a, b = part.split(",")
                out.add((int(a), int(b)))
        return out

    DVE_CHUNKS = _parse(_dve)
    GPS_CHUNKS = _parse(_gps)
    # per-chunk column split: DVE takes the LAST `SPLIT_COLS` columns of each
    # non-diag ACT chunk (short DVE ops so diag masks aren't queued behind
    # long ones), ScalarE the rest
    SPLIT_COLS = int(_os.environ.get("SPLIT_COLS", "54"))
    NORM_GPS = _os.environ.get("NORM_GPS", "0") == "1"
    # PV steps drained between an offloaded chunk's matmuls and its exp
    # emission: their recip/norm ops enter the DVE FIFO before the exp, so
    # the exp's wait-for-matmuls doesn't head-of-line-block them
    EX_DELAY = int(_os.environ.get("EX_DELAY", "0"))
    # NORM_HOST: the device ships the unnormalized numerator plus the
    # denominator column (VW wide), and the host performs the final divide.
    # (GPSIMD cannot read PSUM, so the acc->SBUF stage stays on DVE either
    # way; host-normalize still drops the reciprocal+multiply.)
    NORM_HOST = _os.environ.get("NORM_HOST", "1") == "1"
    # MASK_GPS: diag masks (SBUF-only tensor_mul) run on the idle GpSimd
    MASK_GPS = _os.environ.get("MASK_GPS", "1") == "1"
    # STAGE_ACT: the acc->SBUF stage runs on ScalarE (activation Copy)
    # instead of DVE; STAGE_BF16: stage+store in bf16 (halves store traffic)
    STAGE_ACT = _os.environ.get("STAGE_ACT", "0") == "1"
    STAGE_BF16 = _os.environ.get("STAGE_BF16", "1") == "1"
    # per-head unit order (steady-state heads): balances exp-column supply
    # against PV-drain bursts at head boundaries
    G_ORDER = [int(x) for x in
               _os.environ.get("G_ORDER", "0,3,1,2").split(",")]
    # head 0 ramps with incremental data needs (g1 needs far less than g3),
    # so its unit order favors load streaming over engine balance
    H0_ORDER = [int(x) for x in
                _os.environ.get("H0_ORDER", "0,1,2,3").split(",")]
    ACT_COST_SCALE = float(_os.environ.get("ACT_COST_SCALE", "0.85"))
    ST_BUFS = int(_os.environ.get("ST_BUFS", "3"))
    ACC_BUFS = int(_os.environ.get("ACC_BUFS", "2"))
    PT_BUFS = int(_os.environ.get("PT_BUFS", "3"))
    MASK_MM = _os.environ.get("MASK_MM", "0") == "1"
    # non-diag chunks whose score matmuls run as fp8e4 DoubleRow (2x PE rate;
    # K carried at hi+lo/16 double-fp8 precision, Q one-sided e4m3).
    # FP8_DIAG_GS: groups whose DIAGONAL blocks also run fp8 (g0's diagonal
    # carries the highest softmax output weight — early rows have few keys
    # and large output norms — so it stays bf16).
    FP8_CHUNKS = _parse(_os.environ.get(
        "FP8_SET", "(1,0);(1,1);"
                   "(2,0);(2,1);(2,2);(2,3);"
                   "(3,0);(3,1);(3,2);(3,3);(3,4);(3,5)"))
    FP8_DIAG_GS = {int(x) for x in
                   _os.environ.get("FP8_DIAG_GS", "1,2,3").split(",")
                   if x.strip()}
    FP8_JS = sorted(
        {j for (g, ci) in FP8_CHUNKS for j in (2 * ci, 2 * ci + 1)
         if j < 4 * g}
        | {4 * g + i for g in FP8_DIAG_GS for i in range(4)}
    )
    FP8_GS = sorted({g for (g, ci) in FP8_CHUNKS} | FP8_DIAG_GS)
    NJ8 = len(FP8_JS)
    J8_COL = {j: i * QB for i, j in enumerate(FP8_JS)}   # khl col offset
    G8_COL = {g: i * GW for i, g in enumerate(FP8_GS)}   # q8 col offset
    # bf16 residual needs: which qt group windows / kt j-blocks still load
    BF16_GS = sorted(
        {g for g in range(NG) for ci in range(2 * g)
         if (g, ci) not in FP8_CHUNKS}
        | {g for g in range(NG) if g not in FP8_DIAG_GS}
    )
    BF16_JS = sorted(
        {j for g in range(NG) for ci in range(2 * g)
         if (g, ci) not in FP8_CHUNKS for j in (2 * ci, 2 * ci + 1)}
        | {4 * g + i for g in range(NG) if g not in FP8_DIAG_GS
           for i in range(4)}
    )
    if FP8_DIAG_GS:
        assert not MASK_MM, "fp8 diag requires the DVE mask path (MASK_MM=0)"
    assert 0 not in FP8_DIAG_GS, "g0 diag must stay bf16 (p0 startup path)"
    _CACHE["mask_mm"] = MASK_MM
    KT_COL = {j: i * QB for i, j in enumerate(BF16_JS)}
    QT_COL = {g: i * GW for i, g in enumerate(BF16_GS)}
    C1E = float(np.log2(np.e) * 128.0 * SCALE)
    C2E = 127.0 * 128.0 - 7.5

    nc = bacc.Bacc("TRN2", target_bir_lowering=False, num_devices=N_CORES)

    qt_d = nc.dram_tensor("qt", [HPC, 128, S], bf16, kind="ExternalInput").ap()
    kt_d = nc.dram_tensor("kt", [HPC, 128, S], bf16, kind="ExternalInput").ap()
    va_d = nc.dram_tensor("va", [HPC, 128, NJ * VW], bf16, kind="ExternalInput").ap()
    # packed startup tile: [negI | lower1 | kt0 0:512 | qt0 0:512] —
    # mask-matmul constants plus everything head-0's g0 diag chunk needs,
    # fetched in a single DMA (per-DMA fixed costs dominate the startup
    # critical path)
    p0_d = nc.dram_tensor("p0", [128, 2 * QB + 2 * GW], bf16,
                          kind="ExternalInput").ap()
    f8e4 = mybir.dt.float8e4
    DRPM = mybir.MatmulPerfMode.DoubleRow
    if NJ8:
        khl_d = nc.dram_tensor("khl", [HPC, 128, 2, NJ8 * QB], f8e4,
                               kind="ExternalInput").ap()
        q8_d = nc.dram_tensor("q8", [HPC, 128, 2, len(FP8_GS) * GW], f8e4,
                              kind="ExternalInput").ap()
    _CACHE["fp8"] = (FP8_JS, FP8_GS)
    _CACHE["norm_host"] = NORM_HOST
    OW = VW if NORM_HOST else D   # output row width
    out_dt = bf16 if STAGE_BF16 else f32
    _CACHE["out_bf16"] = STAGE_BF16
    # partition-major out layout: out[h, p, j*OW + d] holds row q = j*128 + p.
    # Stores become fully contiguous per partition (4*OW*2B = 1032B runs at
    # full DMA bus rate vs 258B rows at half rate); host un-shuffles.
    out_d = nc.dram_tensor("out", [HPC, 128, NJ * OW], out_dt,
                           kind="ExternalOutput").ap()

    with TileContext(nc) as tc:
        with (
            tc.tile_pool(name="consts", bufs=1) as consts,
            tc.tile_pool(name="io", bufs=3) as io,
            tc.tile_pool(name="pt", bufs=PT_BUFS) as ptp,
            tc.tile_pool(name="ob", bufs=4) as obp,
            tc.tile_pool(name="rr", bufs=4) as rrp,
            tc.tile_pool(name="st", bufs=ST_BUFS, space="PSUM") as stp,
            tc.tile_pool(name="acc", bufs=ACC_BUFS, space="PSUM") as accp,
        ):
            # packed startup tile [negI | lower1 | kt0 0:512 | qt0 0:512]:
            # one DMA with one completion-sem covers everything unit (h0, g0)
            # consumes. negI/lower1 implement causal masking INSIDE the score
            # accumulation: st[k,q] += -60000*[k>q] via one extra 128-col
            # matmul per diag block, so exp yields exact zeros above the
            # diagonal and no post-exp mask op exists on any engine.
            p0_sb = consts.tile([128, 2 * QB + 2 * GW], bf16, name="p0_sb")
            nc.sync.dma_start(out=p0_sb[:, :], in_=p0_d[:, :])
            negi_sb = p0_sb[:, 0:QB]
            low1_sb = p0_sb[:, QB:2 * QB]
            p0_kt = p0_sb[:, 2 * QB:2 * QB + GW]
            p0_qt = p0_sb[:, 2 * QB + GW:2 * QB + 2 * GW]
            # HAM warm-up: PE idles ~2us at start waiting for the first DMA
            # anyway; dummy matmuls on memset SBUF keep the PE activity
            # monitor busy so real work starts fast (real-HW; near-neutral in
            # sim). PSUM target is overwritten by start=True.
            warm_in = consts.tile([128, VW], bf16, name="warm_in")
            nc.vector.memset(warm_in[:, :], 0.0)
            # hoist the ACT table load (real-HW ~2.7us incl drain) into the
            # startup DMA window via a dummy 1-col activation
            warm_ex = consts.tile([128, 1], bf16, name="warm_ex")
            nc.scalar.activation(warm_ex[:, :], warm_in[:, 0:1], EXP,
                                 scale=SCALE)
            warm_acc = accp.tile([128, VW], f32, tag="acc", name="warm_acc")
            for _ in range(16):
                nc.tensor.matmul(
                    warm_acc[:, :], lhsT=warm_in[:, 0:QB],
                    rhs=warm_in[:, 0:VW], start=True, stop=True,
                )

            def load_head(h):
                """bf16 qt/kt tiles hold only the group-windows / j-blocks
                still computed in bf16 (KT_COL/QT_COL give their offsets);
                fp8 khl/q8 carry everything else. All loads issue on SP
                (HWDGE) so the Pool engine stays free for the diag masks."""
                nq = max(len(BF16_GS), 1)
                nk = max(len(BF16_JS), 1)
                qt_sb = io.tile([128, nq * GW], bf16, tag="qt", name=f"qt{h}")
                kt_sb = io.tile([128, nk * QB], bf16, tag="kt", name=f"kt{h}")
                va_sb = io.tile([128, NJ * VW], bf16, tag="va", name=f"va{h}")
                if NJ8:
                    khl_sb = io.tile([128, 2, NJ8 * QB], f8e4, tag="khl",
                                     name=f"khl{h}")
                    q8_sb = io.tile([128, 2, len(FP8_GS) * GW], f8e4,
                                    tag="q8", name=f"q8{h}")
                else:
                    khl_sb = q8_sb = None
                m = (NJ * VW) // 2
                skip_p0 = h == 0
                # p0 fully covers head 0's bf16 needs when the bf16 residue
                # is exactly g0's diagonal (all-fp8 nd + fp8 diag g1..g3)
                p0_covers = (skip_p0 and BF16_JS == [0, 1, 2, 3]
                             and BF16_GS == [0])

                def kt_runs():
                    runs = []
                    for j in BF16_JS:
                        if runs and runs[-1][1] == j:
                            runs[-1][1] = j + 1
                        else:
                            runs.append([j, j + 1])
                    return runs

                def kt_piece(j0, j1):
                    if j0 >= j1:
                        return
                    nc.sync.dma_start(
                        out=kt_sb[:, KT_COL[j0]:KT_COL[j0] + (j1 - j0) * QB],
                        in_=kt_d[h, :, j0 * QB:j1 * QB])

                def qt_piece(g):
                    nc.sync.dma_start(
                        out=qt_sb[:, QT_COL[g]:QT_COL[g] + GW],
                        in_=qt_d[h, :, g * GW:(g + 1) * GW])

                def q8_piece(g):
                    c = G8_COL[g]
                    nc.sync.dma_start(out=q8_sb[:, :, c:c + GW],
                                      in_=q8_d[h, :, :, c:c + GW])

                if not p0_covers:
                    for j0, j1 in kt_runs():
                        kt_piece(j0, j1)
                    for g in BF16_GS:
                        if skip_p0 and g == 0:
                            continue
                        qt_piece(g)
                # first-use order for unit order g0,g3,...: va first half
                # feeds g0's PV; khl/q8 g3 next; the rest streams under
                # compute
                nc.sync.dma_start(out=va_sb[:, 0:m], in_=va_d[h, :, 0:m])
                if NJ8:
                    half = (NJ8 * QB) // 2
                    nc.sync.dma_start(out=khl_sb[:, :, 0:half],
                                      in_=khl_d[h, :, :, 0:half])
                    nc.sync.dma_start(out=khl_sb[:, :, half:],
                                      in_=khl_d[h, :, :, half:])
                    if 3 in FP8_GS:
                        q8_piece(3)
                nc.sync.dma_start(out=va_sb[:, m:], in_=va_d[h, :, m:])
                for g in FP8_GS:
                    if g != 3:
                        q8_piece(g)
                return qt_sb, kt_sb, va_sb, khl_sb, q8_sb

            def s_chunks(u):
                """Per chunk of unit u: (mm_closures_with_cost, exp_closure,
                act_cost). S^T matmuls land in bank-aligned PSUM chunks
                (<=3 banks), one exp per chunk, diag masks after the exp."""
                h, g, bufs, pt_sb, offs, chunks = u
                qt_sb, kt_sb = bufs[0], bufs[1]
                khl_sb, q8_sb = bufs[3], bufs[4]
                qhi = GW * (g + 1)
                for ci, (col0, entries, cw) in enumerate(chunks):
                    st = stp.tile([128, 1024], f32, tag="st",
                                  name=f"st{h}g{g}c{ci}")
                    mms = []
                    mmcost = 0
                    fp8c = (g, ci) in FP8_CHUNKS and all(
                        j < 4 * g for (j, _q, _o, _w) in entries
                    )
                    for (j, qlo, off, w) in entries:
                        if fp8c or (j >= 4 * g and g in FP8_DIAG_GS):
                            def mm(j=j, qlo=qlo, off=off, w=w, st=st, g=g):
                                c8 = G8_COL[g] + (qlo - GW * g)
                                nc.tensor.matmul(
                                    st[:, off:off + w],
                                    lhsT=khl_sb[
                                        :, :, J8_COL[j]:J8_COL[j] + QB
                                    ],
                                    rhs=q8_sb[:, :, c8:c8 + w],
                                    start=True, stop=True, perf_mode=DRPM,
                                )
                            mms.append(mm)
                            mmcost += w // 2 + 8
                        elif j >= 4 * g and MASK_MM:
                            # diag block: accumulate -60000 above the diagonal
                            # in the same PSUM group (masking via the PE)
                            def mm(j=j, qlo=qlo, off=off, w=w, st=st, g=g):
                                cq = QT_COL[g] + (qlo - GW * g)
                                nc.tensor.matmul(
                                    st[:, off:off + w],
                                    lhsT=kt_sb[:, KT_COL[j]:KT_COL[j] + QB],
                                    rhs=qt_sb[:, cq:cq + w],
                                    start=True, stop=False,
                                )
                                nc.tensor.matmul(
                                    st[:, off:off + QB],
                                    lhsT=negi_sb[:, :],
                                    rhs=low1_sb[:, :],
                                    start=False, stop=True,
                                    skip_group_check=True,
                                )
                            mms.append(mm)
                            mmcost += w + QB + 16
                        else:
                            def mm(j=j, qlo=qlo, off=off, w=w, st=st, g=g):
                                cq = QT_COL[g] + (qlo - GW * g)
                                nc.tensor.matmul(
                                    st[:, off:off + w],
                                    lhsT=kt_sb[:, KT_COL[j]:KT_COL[j] + QB],
                                    rhs=qt_sb[:, cq:cq + w],
                                    start=True, stop=True,
                                )
                            mms.append(mm)
                            mmcost += w + 8

                    # offload a slice of the exp work to the idle VectorE /
                    # GpSimd engines via the exp2 bit-trick: bf16 bits =
                    # rint(x*log2e*128 + C2E) (one tensor_scalar, f32->int16
                    # convert aliased over the bf16 tile). ~1.3% element
                    # error, bias-centered so the softmax output error stays
                    # small. (the split-all final unit keeps everything on
                    # ScalarE)
                    final = g == 0 and len(chunks) > 1
                    nd_chunk = all(j < 4 * g for (j, _q, _o, _w) in entries)
                    eng = "act"
                    if not final and (nd_chunk or not MASK_MM):
                        # (with MASK_MM the bit-trick's f32->int16 convert
                        # would wrap on the -60000 masked scores, so diag
                        # chunks are only offloadable on the DVE-mask path)
                        if (g, ci) in DVE_CHUNKS:
                            eng = "dve"
                        elif (g, ci) in GPS_CHUNKS:
                            eng = "gps"

                    sl = SPLIT_COLS if (eng == "act" and nd_chunk and not final
                                        and SPLIT_COLS < cw) else 0

                    def ex(col0=col0, cw=cw, st=st, entries=entries, eng=eng,
                           sl=sl):
                        if eng == "act":
                            nc.scalar.activation(
                                pt_sb[:, col0:col0 + cw - sl],
                                st[:, 0:cw - sl], EXP, scale=SCALE,
                            )
                            if sl:
                                nc.vector.tensor_scalar(
                                    pt_sb[
                                        :, col0 + cw - sl:col0 + cw
                                    ].bitcast(i16),
                                    st[:, cw - sl:cw], C1E, C2E,
                                    mybir.AluOpType.mult, mybir.AluOpType.add,
                                )
                        else:
                            e = nc.vector if eng == "dve" else nc.gpsimd
                            e.tensor_scalar(
                                pt_sb[:, col0:col0 + cw].bitcast(i16),
                                st[:, 0:cw], C1E, C2E,
                                mybir.AluOpType.mult, mybir.AluOpType.add,
                            )
                        if not MASK_MM:
                            # negi_sb slot holds the inclusive upper-tri mask
                            # in this mode (host-selected)
                            me = nc.gpsimd if MASK_GPS else nc.vector
                            for (j, qlo, off, w) in entries:
                                if j >= 4 * g:  # diag: zero where k > q
                                    me.tensor_mul(
                                        pt_sb[:, col0 + off:col0 + off + QB],
                                        pt_sb[:, col0 + off:col0 + off + QB],
                                        negi_sb[:, :],
                                    )
                    if eng == "act":
                        ecost = int(2 * (cw - sl + 222) * ACT_COST_SCALE)
                    elif eng == "dve":
                        ecost = int(2.5 * cw) + 300
                    else:
                        ecost = int(3.33 * cw) + 700
                    yield mms, mmcost, ex, eng, ecost

            def pv_steps(u, split_store=False, store_eng=None):
                """(pe_cost, closure) steps: PV accumulation matmuls +
                stage + store for unit u. acc tiles hold HALF a unit
                (2 q-blocks, exactly 1 PSUM bank) so 3 score chunks + 2 accs
                fit the 8 banks. Staging happens per half (one 2*OW-col op),
                the store once per unit (split_store also stores the first
                half early for the endgame tail)."""
                h, g, bufs, pt_sb, offs, _chunks = u
                va_sb = bufs[2]
                if store_eng is None:
                    store_eng = nc.sync
                o_grp = obp.tile([128, 4 * OW], out_dt, tag="obg",
                                 name=f"og{h}g{g}")
                acc = None
                for c in range(4):
                    Q = 4 * g + c
                    qlo_c = GW * g + QB * c
                    if c % 2 == 0:
                        acc = accp.tile([128, 2 * VW], f32, tag="acc",
                                        name=f"acc{h}g{g}c{c}")
                    a0 = (c % 2) * VW
                    for j in range(Q + 1):
                        qlo_j, col_j = offs[j]
                        off = col_j + (qlo_c - qlo_j)

                        def step(j=j, Q=Q, off=off, acc=acc, a0=a0):
                            nc.tensor.matmul(
                                acc[:, a0:a0 + VW],
                                lhsT=pt_sb[:, off:off + QB],
                                rhs=va_sb[:, j * VW:(j + 1) * VW],
                                start=(j == 0), stop=(j == Q),
                            )
                        yield 300, step

                    if c % 2 == 0:
                        continue

                    def fin(c=c, acc=acc, o_grp=o_grp):
                        ob0 = (c - 1) * OW
                        if NORM_HOST:
                            # ship numerator+denominator (only ACT/DVE can
                            # read PSUM); host divides. One 2*OW-wide stage
                            # per half-unit.
                            if STAGE_ACT:
                                nc.scalar.activation(
                                    o_grp[:, ob0:ob0 + 2 * OW],
                                    acc[:, :],
                                    mybir.ActivationFunctionType.Copy,
                                    scale=1.0,
                                )
                            else:
                                nc.vector.tensor_copy(
                                    o_grp[:, ob0:ob0 + 2 * OW],
                                    acc[:, :],
                                )
                        else:
                            for cc in (c - 1, c):
                                a0 = (cc % 2) * VW
                                r = rrp.tile([128, 1], f32, tag="r",
                                             name=f"r{h}g{g}c{cc}")
                                nc.vector.reciprocal(
                                    r[:, :], acc[:, a0 + D:a0 + D + 1])
                                nc.vector.tensor_scalar_mul(
                                    o_grp[:, cc * OW:(cc + 1) * OW],
                                    acc[:, a0:a0 + D], r[:, :],
                                )
                        if split_store and c == 1:
                            store_eng.dma_start(
                                out=out_d[h, :,
                                          4 * g * OW:(4 * g + 2) * OW],
                                in_=o_grp[:, 0:2 * OW])
                        elif split_store and c == 3:
                            store_eng.dma_start(
                                out=out_d[h, :,
                                          (4 * g + 2) * OW:(4 * g + 4) * OW],
                                in_=o_grp[:, 2 * OW:4 * OW])
                        elif not split_store and c == 3:
                            store_eng.dma_start(
                                out=out_d[h, :,
                                          4 * g * OW:(4 * g + 4) * OW],
                                in_=o_grp[:, :])
                    yield 0, fin

            def make_unit(h, g, bufs, split_all=False):
                # Chunk layout: non-diag js in twos (512 each, bank aligned),
                # then the diag chunk packed 512+384 | 256+128 into 2.5 banks.
                # chunks: list of (pt_col0, [(j, qlo, off_in_chunk, w)], width)
                chunks = []
                col = 0
                nd = 4 * g  # non-diagonal k-blocks
                for i0 in range(0, nd, 2):
                    entries = [
                        (j, GW * g, (j - i0) * GW, GW)
                        for j in range(i0, min(i0 + 2, nd))
                    ]
                    cw = len(entries) * GW
                    chunks.append((col, entries, cw))
                    col += cw
                d0 = 4 * g
                if split_all:
                    # per-j chunks (used for the final unit so its PV can
                    # begin before the whole diagonal chunk is exp'd)
                    for i, w in enumerate((512, 384, 256, 128)):
                        chunks.append(
                            (col, [(d0 + i, QB * (d0 + i), 0, w)], w)
                        )
                        col += w
                else:
                    chunks.append((col, [
                        (d0, QB * d0, 0, 512),
                        (d0 + 1, QB * (d0 + 1), 512, 384),
                    ], 896))
                    col += 896
                    chunks.append((col, [
                        (d0 + 2, QB * (d0 + 2), 0, 256),
                        (d0 + 3, QB * (d0 + 3), 256, 128),
                    ], 384))
                    col += 384
                offs = {}
                for col0, entries, _ in chunks:
                    for (j, qlo, off, _w) in entries:
                        offs[j] = (qlo, col0 + off)
                pt_sb = ptp.tile(
                    [128, 12 * GW + 1280], bf16, tag="pt", name=f"pt{h}g{g}"
                )
                return (h, g, bufs, pt_sb, offs, chunks)

            # Global clock-based pacing: emit exp chunks on each exp-engine's
            # schedule, fill PE's spare time from a queue of pending PV work.
            # Clocks in PE cycles @2.4GHz; ACT cycles count double, DVE 2.5x,
            # GPS 3.33x. chunk_end tracks modeled exp completions so chunk
            # k's matmuls are delayed until chunk k-3's PSUM slot frees
            # (3-slot st pool) with PV filler emitted in the meantime.
            pe_clock = 0.0
            eng_clock = {"act": 0.0, "dve": 0.0, "gps": 0.0}
            SEMC = 240.0  # ~100ns handoff latency in PE cycles
            chunk_end = []
            pvq = []  # list of (unit_idx, pe_cost, closure), FIFO
            qi = 0

            def drain_pv(upto_unit=None, clock_limit=None, nsteps=None):
                nonlocal qi, pe_clock
                done = 0
                while qi < len(pvq):
                    uidx, cost, fn = pvq[qi]
                    if upto_unit is not None and uidx > upto_unit:
                        break
                    if clock_limit is not None and pe_clock >= clock_limit:
                        break
                    if nsteps is not None and done >= nsteps:
                        break
                    fn()
                    pe_clock += cost
                    qi += 1
                    done += 1

            head_bufs = [None] * HPC
            head_bufs[0] = load_head(0)
            uidx = 0
            for h in range(HPC):
                if h + 1 < HPC:
                    head_bufs[h + 1] = load_head(h + 1)
                if h == HPC - 1:
                    gs = [int(x) for x in _os.environ.get(
                        "LAST_ORDER", "2,3,1,0").split(",")]
                elif h == 0:
                    gs = H0_ORDER
                else:
                    gs = G_ORDER
                for g in gs:
                    # pt pool has PT_BUFS slots: before unit uidx's first exp
                    # can run, unit uidx-PT_BUFS's PV (the slot's previous
                    # holder's reader) must be fully emitted on PE's stream.
                    drain_pv(upto_unit=uidx - PT_BUFS)
                    last = uidx == NG * HPC - 1
                    bufs_u = head_bufs[h]
                    if h == 0 and g == 0:
                        bufs_u = (p0_qt, p0_kt) + tuple(bufs_u[2:])
                    u = make_unit(h, g, bufs_u, split_all=last)
                    if last:
                        # endgame: per-j chunks; leftover PV of previous units
                        # drains under the first exp; each own PV chain goes
                        # right after the per-j exp it depends on
                        own = list(pv_steps(u, split_store=True))
                        oi = 0
                        for ci, (mms, mmcost, ex, eng, ecost) in enumerate(
                            s_chunks(u)
                        ):
                            if ci == 0:
                                drain_pv()
                            for mm in mms:
                                mm()
                            ex()
                            for _ in range(ci + 2):  # ~c+1 matmuls + fin
                                if oi < len(own):
                                    own[oi][1]()
                                    oi += 1
                        while oi < len(own):
                            own[oi][1]()
                            oi += 1
                        continue
                    for mms, mmcost, ex, eng, ecost in s_chunks(u):
                        # give PE filler work until this chunk's exp engine
                        # and its PSUM slot are about to be available
                        target = eng_clock[eng]
                        if len(chunk_end) >= ST_BUFS:
                            target = max(target, chunk_end[-ST_BUFS])
                        drain_pv(clock_limit=target - mmcost)
                        for mm in mms:
                            mm()
                        pe_clock += mmcost
                        if eng != "act" and EX_DELAY:
                            drain_pv(nsteps=EX_DELAY)
                        ex()
                        e_end = max(eng_clock[eng], pe_clock + SEMC) + ecost
                        eng_clock[eng] = e_end
                        chunk_end.append(e_end)
                    pvq.extend(
                        (uidx, cost, fn) for cost, fn in pv_steps(u)
                    )
                    uidx += 1
            drain_pv()

    nc.compile()
    return nc


_F8 = ml_dtypes.float8_e4m3


def _e4m3(x):
    return np.clip(x, -240.0, 240.0).astype(_F8)


def _prep_core(q, k, v):
    """q,k,v: [HPC, S, D] f32 for one core -> device input dict."""
    qtf = np.ascontiguousarray(q.transpose(0, 2, 1))  # [HPC, d, S] f32
    ktf = np.ascontiguousarray(k.transpose(0, 2, 1))
    qt = qtf.astype(_BF16)
    kt = ktf.astype(_BF16)
    va = np.empty((HPC, S, VW), dtype=np.float32)
    va[:, :, :D] = v
    va[:, :, D] = 1.0
    # [HPC, S, VW] -> [HPC, 128, NJ*VW]  with [p, j*VW+c] = va[j*128+p, c]
    va = np.ascontiguousarray(
        va.reshape(HPC, NJ, QB, VW).transpose(0, 2, 1, 3)
    ).reshape(HPC, QB, NJ * VW).astype(_BF16)
    m = {"qt": qt, "kt": kt, "va": va}
    fp8_js, fp8_gs = _CACHE.get("fp8", ([], []))
    if fp8_js:
        # K at double-fp8 (hi + lo/16), Q one-sided e4m3 (+ /16 copy for the
        # DoubleRow second slot)
        kcols = np.concatenate(
            [ktf[:, :, j * QB:(j + 1) * QB] for j in fp8_js], axis=2
        )
        k_hi = _e4m3(kcols)
        k_lo = _e4m3((kcols - k_hi.astype(np.float32)) * 16.0)
        m["khl"] = np.stack([k_hi, k_lo], axis=2)  # [HPC, 128, 2, NJ8*QB]
        qcols = np.concatenate(
            [qtf[:, :, g * GW:(g + 1) * GW] for g in fp8_gs], axis=2
        )
        q8 = _e4m3(qcols)
        q8s = (q8.astype(np.float32) / 16.0).astype(_F8)
        m["q8"] = np.stack([q8, q8s], axis=2)      # [HPC, 128, 2, NG8*GW]
    return m


def _run(query, key, value, trace=False):
    from concourse import bass_utils

    if "nc" not in _CACHE:
        _CACHE["nc"] = _build()
    nc = _CACHE["nc"]

    q = np.asarray(query, dtype=np.float32).reshape(B * H, S, D)
    k = np.asarray(key, dtype=np.float32).reshape(B * H, S, D)
    v = np.asarray(value, dtype=np.float32).reshape(B * H, S, D)
    if _CACHE["mask_mm"]:
        negi = (-60000.0 * np.eye(128, dtype=np.float32)).astype(_BF16)
    else:
        negi = np.triu(np.ones((128, 128), dtype=np.float32)).astype(_BF16)
    low1 = np.tril(np.ones((128, 128), dtype=np.float32), -1).astype(_BF16)

    in_maps = []
    for c in range(N_CORES):
        sl = slice(c * HPC, (c + 1) * HPC)
        m = _prep_core(q[sl], k[sl], v[sl])
        # packed startup tile: [negI | lower1 | kt0 0:512 | qt0 0:512]
        m["p0"] = np.concatenate(
            [negi, low1, m["kt"][0][:, :512], m["qt"][0][:, :512]], axis=1
        )
        in_maps.append(m)

    res = bass_utils.run_bass_kernel_spmd(
        nc, in_maps, core_ids=list(range(N_CORES)), trace=trace
    )
    outs = [res.results[c]["out"] for c in range(N_CORES)]
    full = np.concatenate(outs, axis=0).astype(np.float32)
    # device layout is partition-major: out[h, p, j*OW + d] = row q = j*128+p
    OW = full.shape[-1] // NJ
    full = full.reshape(B * H, QB, NJ, OW).transpose(0, 2, 1, 3)
    full = np.ascontiguousarray(full).reshape(B * H, S, OW)
    if _CACHE.get("norm_host"):
        full = full[..., :D] / full[..., D:D + 1]
    full = full.reshape(B, H, S, D)
    return full, res


def kernel(query, key, value, mask=None):
    """Full inputs in, full output out. `mask` is the causal mask from
    setup_inputs (strictly-upper-triangular True = disallowed); causality is
    implemented structurally so the tensor itself is not consumed."""
    out, _ = _run(query, key, value, trace=False)
    return out



# revision 14
# speedup vs baseline: 1.2660x; 1.2660x over previous
"""Causal multi-head attention on 8 TRN2 NeuronCores.

Problem: B=4, H=16, S=2048, D=128 fp32 causal attention.
Sharding: batch*heads (64) split 8-per-core across the 8 cores; each core
computes its heads fully independently (no collectives).

Per-core kernel strategy (f32 accumulation):
  - scores computed TRANSPOSED: S^T[k,q] = K_j @ Q^T per (k-block j of 128,
    q-group g of 512), causal blocks only, into 2-bank PSUM chunks (<=1024)
    with a 3-deep chunk pipeline
  - ALL non-diagonal score blocks of groups g2/g3 run as fp8e4 DoubleRow
    matmuls at 2x PE rate (cost-model 0.5 cycles/row): the stationary K side
    carries hi + lo/16 double-fp8 (~8 mantissa bits), the moving Q side is
    one-sided e4m3 with a /16 second slot. Host pre-quantizes Q/K; measured
    output rel err ~1.0% vs the 2e-2 gate. fp8 on g0/g1/diagonal blocks is
    deliberately avoided: early rows have few keys and large output norms,
    so they dominate the error weighting.
  - exp mostly on ScalarE (PSUM -> SBUF bf16, 1/sqrt(D) folded into the
    activation scale); four late chunks per head offloaded to VectorE via an
    exp2 bit-trick (one tensor_scalar: bf16 bits = rint(x*log2e*128 +
    127*128 - 7.5), f32->int16 convert aliased over the bf16 tile)
  - diagonal 128x128 blocks masked with a constant triangular tile on DVE
  - PV: out[q,:] = P^T_slice.T @ [V_j | ones]; the appended ones column
    yields the softmax denominator in the same accumulation. Output lands
    directly in [q, d] layout.
  - normalize with VectorE reciprocal + per-partition tensor_scalar mult
  - packed single-DMA startup tile (mask consts + first K/Q block); per-head
    unit order g0,g3,g1,g2 balances exp-column supply against PV drains;
    per-engine pacing clocks keep the PE fed with PV filler work
"""

import sys

import numpy as np
import ml_dtypes

for _p in ("/opt/trn_rl_repo", "/root/.axon_site/_ro/trn_rl_repo"):
    try:
        import concourse  # noqa: F401
        break
    except ImportError:
        if _p not in sys.path:
            sys.path.append(_p)

B, H, S, D = 4, 16, 2048, 128
N_CORES = 8
HPC = (B * H) // N_CORES  # heads per core = 8
QB = 128                  # q/k block
GW = 512                  # q group width
NG = S // GW              # 4 groups per head
NJ = S // QB              # 16 k blocks
VW = D + 1                # V augmented with ones column = 129
SCALE = 1.0 / float(np.sqrt(D))

_BF16 = ml_dtypes.bfloat16

_CACHE = {}


def _build():
    import concourse.bass as bass  # noqa: F401
    import concourse.mybir as mybir
    from concourse import bacc
    from concourse.tile import TileContext

    f32 = mybir.dt.float32
    bf16 = mybir.dt.bfloat16
    i16 = mybir.dt.int16
    EXP = mybir.ActivationFunctionType.Exp
    # chunks routed to the bit-trick exp on VectorE / GpSimd: (g, chunk_idx).
    # Late chunks only — their PV consumption comes last in each q-chain, so
    # the slower engines' latency hides behind ScalarE's pipeline.
    import os as _os
    _dve = _os.environ.get("DVE_SET",
                           "(3,0);(3,1);(3,2);(3,3);(3,4);(3,5)")
    _gps = _os.environ.get("GPS_SET", "")

    def _parse(s):
        out = set()
        for part in s.split(";"):
            part = part.strip().strip("()")
            if part:
                a, b = part.split(",")
                out.add((int(a), int(b)))
        return out

    DVE_CHUNKS = _parse(_dve)
    GPS_CHUNKS = _parse(_gps)
    # per-chunk column split: DVE takes the LAST `SPLIT_COLS` columns of each
    # non-diag ACT chunk (short DVE ops so diag masks aren't queued behind
    # long ones), ScalarE the rest
    SPLIT_COLS = int(_os.environ.get("SPLIT_COLS", "54"))
    NORM_GPS = _os.environ.get("NORM_GPS", "0") == "1"
    # PV steps drained between an offloaded chunk's matmuls and its exp
    # emission: their recip/norm ops enter the DVE FIFO before the exp, so
    # the exp's wait-for-matmuls doesn't head-of-line-block them
    EX_DELAY = int(_os.environ.get("EX_DELAY", "0"))
    # NORM_HOST: the device ships the unnormalized numerator plus the
    # denominator column (VW wide), and the host performs the final divide.
    # (GPSIMD cannot read PSUM, so the acc->SBUF stage stays on DVE either
    # way; host-normalize still drops the reciprocal+multiply.)
    NORM_HOST = _os.environ.get("NORM_HOST", "1") == "1"
    # MASK_GPS: diag masks (SBUF-only tensor_mul) run on the idle GpSimd
    MASK_GPS = _os.environ.get("MASK_GPS", "1") == "1"
    # STAGE_ACT: the acc->SBUF stage runs on ScalarE (activation Copy)
    # instead of DVE; STAGE_BF16: stage+store in bf16 (halves store traffic)
    STAGE_ACT = _os.environ.get("STAGE_ACT", "0") == "1"
    STAGE_BF16 = _os.environ.get("STAGE_BF16", "1") == "1"
    # per-head unit order (steady-state heads): balances exp-column supply
    # against PV-drain bursts at head boundaries
    G_ORDER = [int(x) for x in
               _os.environ.get("G_ORDER", "0,3,1,2").split(",")]
    # head 0 ramps with incremental data needs (g1 needs far less than g3),
    # so its unit order favors load streaming over engine balance
    H0_ORDER = [int(x) for x in
                _os.environ.get("H0_ORDER", "0,1,2,3").split(",")]
    ACT_COST_SCALE = float(_os.environ.get("ACT_COST_SCALE", "0.85"))
    ST_BUFS = int(_os.environ.get("ST_BUFS", "3"))
    ACC_BUFS = int(_os.environ.get("ACC_BUFS", "2"))
    PT_BUFS = int(_os.environ.get("PT_BUFS", "3"))
    MASK_MM = _os.environ.get("MASK_MM", "1") == "1"
    # non-diag chunks whose score matmuls run as fp8e4 DoubleRow (2x PE rate;
    # K carried at hi+lo/16 double-fp8 precision, Q one-sided e4m3).
    # FP8_DIAG_GS: groups whose DIAGONAL blocks also run fp8 (g0's diagonal
    # carries the highest softmax output weight — early rows have few keys
    # and large output norms — so it stays bf16).
    FP8_CHUNKS = _parse(_os.environ.get(
        "FP8_SET", "(1,0);(1,1);"
                   "(2,0);(2,1);(2,2);(2,3);"
                   "(3,0);(3,1);(3,2);(3,3);(3,4);(3,5)"))
    FP8_DIAG_GS = {int(x) for x in
                   _os.environ.get("FP8_DIAG_GS", "1,2,3").split(",")
                   if x.strip()}
    FP8_JS = sorted(
        {j for (g, ci) in FP8_CHUNKS for j in (2 * ci, 2 * ci + 1)
         if j < 4 * g}
        | {4 * g + i for g in FP8_DIAG_GS for i in range(4)}
    )
    FP8_GS = sorted({g for (g, ci) in FP8_CHUNKS} | FP8_DIAG_GS)
    NJ8 = len(FP8_JS)
    J8_COL = {j: i * QB for i, j in enumerate(FP8_JS)}   # khl col offset
    G8_COL = {g: i * GW for i, g in enumerate(FP8_GS)}   # q8 col offset
    # bf16 residual needs: which qt group windows / kt j-blocks still load
    BF16_GS = sorted(
        {g for g in range(NG) for ci in range(2 * g)
         if (g, ci) not in FP8_CHUNKS}
        | {g for g in range(NG) if g not in FP8_DIAG_GS}
    )
    BF16_JS = sorted(
        {j for g in range(NG) for ci in range(2 * g)
         if (g, ci) not in FP8_CHUNKS for j in (2 * ci, 2 * ci + 1)}
        | {4 * g + i for g in range(NG) if g not in FP8_DIAG_GS
           for i in range(4)}
    )
    assert 0 not in FP8_DIAG_GS, "g0 diag must stay bf16 (p0 startup path)"
    _CACHE["mask_mm"] = MASK_MM
    KT_COL = {j: i * QB for i, j in enumerate(BF16_JS)}
    QT_COL = {g: i * GW for i, g in enumerate(BF16_GS)}
    C1E = float(np.log2(np.e) * 128.0 * SCALE)
    C2E = 127.0 * 128.0 - 7.5

    nc = bacc.Bacc("TRN2", target_bir_lowering=False, num_devices=N_CORES)

    qt_d = nc.dram_tensor("qt", [HPC, 128, S], bf16, kind="ExternalInput").ap()
    kt_d = nc.dram_tensor("kt", [HPC, 128, S], bf16, kind="ExternalInput").ap()
    va_d = nc.dram_tensor("va", [HPC, 128, NJ * VW], bf16, kind="ExternalInput").ap()
    # packed startup tile: [negI | lower1 | kt0 0:512 | qt0 0:512] —
    # mask-matmul constants plus everything head-0's g0 diag chunk needs,
    # fetched in a single DMA (per-DMA fixed costs dominate the startup
    # critical path)
    p0_d = nc.dram_tensor("p0", [128, 2 * QB + 2 * GW], bf16,
                          kind="ExternalInput").ap()
    f8e4 = mybir.dt.float8e4
    DRPM = mybir.MatmulPerfMode.DoubleRow
    if NJ8:
        khl_d = nc.dram_tensor("khl", [HPC, 128, 2, NJ8 * QB], f8e4,
                               kind="ExternalInput").ap()
        q8_d = nc.dram_tensor("q8", [HPC, 128, 2, len(FP8_GS) * GW], f8e4,
                              kind="ExternalInput").ap()
    _CACHE["fp8"] = (FP8_JS, FP8_GS)
    _CACHE["norm_host"] = NORM_HOST
    OW = VW if NORM_HOST else D   # output row width
    out_dt = bf16 if STAGE_BF16 else f32
    _CACHE["out_bf16"] = STAGE_BF16
    # partition-major out layout: out[h, p, j*OW + d] holds row q = j*128 + p.
    # Stores become fully contiguous per partition (4*OW*2B = 1032B runs at
    # full DMA bus rate vs 258B rows at half rate); host un-shuffles.
    out_d = nc.dram_tensor("out", [HPC, 128, NJ * OW], out_dt,
                           kind="ExternalOutput").ap()

    with TileContext(nc) as tc:
        with (
            tc.tile_pool(name="consts", bufs=1) as consts,
            tc.tile_pool(name="io", bufs=3) as io,
            tc.tile_pool(name="pt", bufs=PT_BUFS) as ptp,
            tc.tile_pool(name="ob", bufs=4) as obp,
            tc.tile_pool(name="rr", bufs=4) as rrp,
            tc.tile_pool(name="st", bufs=ST_BUFS, space="PSUM") as stp,
            tc.tile_pool(name="acc", bufs=ACC_BUFS, space="PSUM") as accp,
        ):
            # packed startup tile [negI | lower1 | kt0 0:512 | qt0 0:512]:
            # one DMA with one completion-sem covers everything unit (h0, g0)
            # consumes. negI/lower1 implement causal masking INSIDE the score
            # accumulation: st[k,q] += -60000*[k>q] via one extra 128-col
            # matmul per diag block, so exp yields exact zeros above the
            # diagonal and no post-exp mask op exists on any engine.
            p0_sb = consts.tile([128, 2 * QB + 2 * GW], bf16, name="p0_sb")
            nc.sync.dma_start(out=p0_sb[:, :], in_=p0_d[:, :])
            negi_sb = p0_sb[:, 0:QB]
            low1_sb = p0_sb[:, QB:2 * QB]
            p0_kt = p0_sb[:, 2 * QB:2 * QB + GW]
            p0_qt = p0_sb[:, 2 * QB + GW:2 * QB + 2 * GW]
            # HAM warm-up: PE idles ~2us at start waiting for the first DMA
            # anyway; dummy matmuls on memset SBUF keep the PE activity
            # monitor busy so real work starts fast (real-HW; near-neutral in
            # sim). PSUM target is overwritten by start=True.
            warm_in = consts.tile([128, VW], bf16, name="warm_in")
            nc.vector.memset(warm_in[:, :], 0.0)
            # hoist the ACT table load (real-HW ~2.7us incl drain) into the
            # startup DMA window via a dummy 1-col activation
            warm_ex = consts.tile([128, 1], bf16, name="warm_ex")
            nc.scalar.activation(warm_ex[:, :], warm_in[:, 0:1], EXP,
                                 scale=SCALE)
            warm_acc = accp.tile([128, VW], f32, tag="acc", name="warm_acc")
            for _ in range(16):
                nc.tensor.matmul(
                    warm_acc[:, :], lhsT=warm_in[:, 0:QB],
                    rhs=warm_in[:, 0:VW], start=True, stop=True,
                )

            def load_head(h):
                """bf16 qt/kt tiles hold only the group-windows / j-blocks
                still computed in bf16 (KT_COL/QT_COL give their offsets);
                fp8 khl/q8 carry everything else. All loads issue on SP
                (HWDGE) so the Pool engine stays free for the diag masks."""
                nq = max(len(BF16_GS), 1)
                nk = max(len(BF16_JS), 1)
                qt_sb = io.tile([128, nq * GW], bf16, tag="qt", name=f"qt{h}")
                kt_sb = io.tile([128, nk * QB], bf16, tag="kt", name=f"kt{h}")
                va_sb = io.tile([128, NJ * VW], bf16, tag="va", name=f"va{h}")
                if NJ8:
                    khl_sb = io.tile([128, 2, NJ8 * QB], f8e4, tag="khl",
                                     name=f"khl{h}")
                    q8_sb = io.tile([128, 2, len(FP8_GS) * GW], f8e4,
                                    tag="q8", name=f"q8{h}")
                else:
                    khl_sb = q8_sb = None
                m = (NJ * VW) // 2
                skip_p0 = h == 0
                # p0 fully covers head 0's bf16 needs when the bf16 residue
                # is exactly g0's diagonal (all-fp8 nd + fp8 diag g1..g3)
                p0_covers = (skip_p0 and BF16_JS == [0, 1, 2, 3]
                             and BF16_GS == [0])

                def kt_runs():
                    runs = []
                    for j in BF16_JS:
                        if runs and runs[-1][1] == j:
                            runs[-1][1] = j + 1
                        else:
                            runs.append([j, j + 1])
                    return runs

                def kt_piece(j0, j1):
                    if j0 >= j1:
                        return
                    nc.sync.dma_start(
                        out=kt_sb[:, KT_COL[j0]:KT_COL[j0] + (j1 - j0) * QB],
                        in_=kt_d[h, :, j0 * QB:j1 * QB])

                def qt_piece(g):
                    nc.sync.dma_start(
                        out=qt_sb[:, QT_COL[g]:QT_COL[g] + GW],
                        in_=qt_d[h, :, g * GW:(g + 1) * GW])

                def q8_piece(g):
                    c = G8_COL[g]
                    nc.sync.dma_start(out=q8_sb[:, :, c:c + GW],
                                      in_=q8_d[h, :, :, c:c + GW])

                if not p0_covers:
                    for j0, j1 in kt_runs():
                        kt_piece(j0, j1)
                    for g in BF16_GS:
                        if skip_p0 and g == 0:
                            continue
                        qt_piece(g)
                # first-use order for unit order g0,g3,...: va first half
                # feeds g0's PV; khl/q8 g3 next; the rest streams under
                # compute
                nc.sync.dma_start(out=va_sb[:, 0:m], in_=va_d[h, :, 0:m])
                if NJ8:
                    half = (NJ8 * QB) // 2
                    nc.sync.dma_start(out=khl_sb[:, :, 0:half],
                                      in_=khl_d[h, :, :, 0:half])
                    nc.sync.dma_start(out=khl_sb[:, :, half:],
                                      in_=khl_d[h, :, :, half:])
                    if 3 in FP8_GS:
                        q8_piece(3)
                nc.sync.dma_start(out=va_sb[:, m:], in_=va_d[h, :, m:])
                for g in FP8_GS:
                    if g != 3:
                        q8_piece(g)
                return qt_sb, kt_sb, va_sb, khl_sb, q8_sb

            def s_chunks(u):
                """Per chunk of unit u: (mm_closures_with_cost, exp_closure,
                act_cost). S^T matmuls land in bank-aligned PSUM chunks
                (<=3 banks), one exp per chunk, diag masks after the exp."""
                h, g, bufs, pt_sb, offs, chunks = u
                qt_sb, kt_sb = bufs[0], bufs[1]
                khl_sb, q8_sb = bufs[3], bufs[4]
                qhi = GW * (g + 1)
                for ci, (col0, entries, cw) in enumerate(chunks):
                    st = stp.tile([128, 1024], f32, tag="st",
                                  name=f"st{h}g{g}c{ci}")
                    mms = []
                    mmcost = 0
                    fp8c = (g, ci) in FP8_CHUNKS and all(
                        j < 4 * g for (j, _q, _o, _w) in entries
                    )
                    for (j, qlo, off, w) in entries:
                        if fp8c or (j >= 4 * g and g in FP8_DIAG_GS):
                            diag_mask = j >= 4 * g and MASK_MM

                            def mm(j=j, qlo=qlo, off=off, w=w, st=st, g=g,
                                   diag_mask=diag_mask):
                                c8 = G8_COL[g] + (qlo - GW * g)
                                nc.tensor.matmul(
                                    st[:, off:off + w],
                                    lhsT=khl_sb[
                                        :, :, J8_COL[j]:J8_COL[j] + QB
                                    ],
                                    rhs=q8_sb[:, :, c8:c8 + w],
                                    start=True, stop=not diag_mask,
                                    perf_mode=DRPM,
                                )
                                if diag_mask:
                                    # causal mask inside the accumulation:
                                    # st[k,q] += -60000*[k>q] on the leading
                                    # 128-col diagonal square of the entry
                                    nc.tensor.matmul(
                                        st[:, off:off + QB],
                                        lhsT=negi_sb[:, :],
                                        rhs=low1_sb[:, :],
                                        start=False, stop=True,
                                        skip_group_check=True,
                                    )
                            mms.append(mm)
                            mmcost += w // 2 + 8
                            if diag_mask:
                                mmcost += QB + 8
                        elif j >= 4 * g and MASK_MM:
                            # diag block: accumulate -60000 above the diagonal
                            # in the same PSUM group (masking via the PE)
                            def mm(j=j, qlo=qlo, off=off, w=w, st=st, g=g):
                                cq = QT_COL[g] + (qlo - GW * g)
                                nc.tensor.matmul(
                                    st[:, off:off + w],
                                    lhsT=kt_sb[:, KT_COL[j]:KT_COL[j] + QB],
                                    rhs=qt_sb[:, cq:cq + w],
                                    start=True, stop=False,
                                )
                                nc.tensor.matmul(
                                    st[:, off:off + QB],
                                    lhsT=negi_sb[:, :],
                                    rhs=low1_sb[:, :],
                                    start=False, stop=True,
                                    skip_group_check=True,
                                )
                            mms.append(mm)
                            mmcost += w + QB + 16
                        else:
                            def mm(j=j, qlo=qlo, off=off, w=w, st=st, g=g):
                                cq = QT_COL[g] + (qlo - GW * g)
                                nc.tensor.matmul(
                                    st[:, off:off + w],
                                    lhsT=kt_sb[:, KT_COL[j]:KT_COL[j] + QB],
                                    rhs=qt_sb[:, cq:cq + w],
                                    start=True, stop=True,
                                )
                            mms.append(mm)
                            mmcost += w + 8

                    # offload a slice of the exp work to the idle VectorE /
                    # GpSimd engines via the exp2 bit-trick: bf16 bits =
                    # rint(x*log2e*128 + C2E) (one tensor_scalar, f32->int16
                    # convert aliased over the bf16 tile). ~1.3% element
                    # error, bias-centered so the softmax output error stays
                    # small. (the split-all final unit keeps everything on
                    # ScalarE)
                    final = g == 0 and len(chunks) > 1
                    nd_chunk = all(j < 4 * g for (j, _q, _o, _w) in entries)
                    eng = "act"
                    if not final and (nd_chunk or not MASK_MM):
                        # (with MASK_MM the bit-trick's f32->int16 convert
                        # would wrap on the -60000 masked scores, so diag
                        # chunks are only offloadable on the DVE-mask path)
                        if (g, ci) in DVE_CHUNKS:
                            eng = "dve"
                        elif (g, ci) in GPS_CHUNKS:
                            eng = "gps"

                    sl = SPLIT_COLS if (eng == "act" and nd_chunk and not final
                                        and SPLIT_COLS < cw) else 0

                    def ex(col0=col0, cw=cw, st=st, entries=entries, eng=eng,
                           sl=sl):
                        if eng == "act":
                            nc.scalar.activation(
                                pt_sb[:, col0:col0 + cw - sl],
                                st[:, 0:cw - sl], EXP, scale=SCALE,
                            )
                            if sl:
                                nc.vector.tensor_scalar(
                                    pt_sb[
                                        :, col0 + cw - sl:col0 + cw
                                    ].bitcast(i16),
                                    st[:, cw - sl:cw], C1E, C2E,
                                    mybir.AluOpType.mult, mybir.AluOpType.add,
                                )
                        else:
                            e = nc.vector if eng == "dve" else nc.gpsimd
                            e.tensor_scalar(
                                pt_sb[:, col0:col0 + cw].bitcast(i16),
                                st[:, 0:cw], C1E, C2E,
                                mybir.AluOpType.mult, mybir.AluOpType.add,
                            )
                        if not MASK_MM:
                            # negi_sb slot holds the inclusive upper-tri mask
                            # in this mode (host-selected)
                            me = nc.gpsimd if MASK_GPS else nc.vector
                            for (j, qlo, off, w) in entries:
                                if j >= 4 * g:  # diag: zero where k > q
                                    me.tensor_mul(
                                        pt_sb[:, col0 + off:col0 + off + QB],
                                        pt_sb[:, col0 + off:col0 + off + QB],
                                        negi_sb[:, :],
                                    )
                    if eng == "act":
                        ecost = int(2 * (cw - sl + 222) * ACT_COST_SCALE)
                    elif eng == "dve":
                        ecost = int(2.5 * cw) + 300
                    else:
                        ecost = int(3.33 * cw) + 700
                    yield mms, mmcost, ex, eng, ecost

            def pv_steps(u, split_store=False, store_eng=None):
                """(pe_cost, closure) steps: PV accumulation matmuls +
                stage + store for unit u. acc tiles hold HALF a unit
                (2 q-blocks, exactly 1 PSUM bank) so 3 score chunks + 2 accs
                fit the 8 banks. Staging happens per half (one 2*OW-col op),
                the store once per unit (split_store also stores the first
                half early for the endgame tail)."""
                h, g, bufs, pt_sb, offs, _chunks = u
                va_sb = bufs[2]
                if store_eng is None:
                    store_eng = nc.sync
                o_grp = obp.tile([128, 4 * OW], out_dt, tag="obg",
                                 name=f"og{h}g{g}")
                acc = None
                for c in range(4):
                    Q = 4 * g + c
                    qlo_c = GW * g + QB * c
                    if c % 2 == 0:
                        acc = accp.tile([128, 2 * VW], f32, tag="acc",
                                        name=f"acc{h}g{g}c{c}")
                    a0 = (c % 2) * VW
                    for j in range(Q + 1):
                        qlo_j, col_j = offs[j]
                        off = col_j + (qlo_c - qlo_j)

                        def step(j=j, Q=Q, off=off, acc=acc, a0=a0):
                            nc.tensor.matmul(
                                acc[:, a0:a0 + VW],
                                lhsT=pt_sb[:, off:off + QB],
                                rhs=va_sb[:, j * VW:(j + 1) * VW],
                                start=(j == 0), stop=(j == Q),
                            )
                        yield 300, step

                    if c % 2 == 0:
                        continue

                    def fin(c=c, acc=acc, o_grp=o_grp):
                        ob0 = (c - 1) * OW
                        if NORM_HOST:
                            # ship numerator+denominator (only ACT/DVE can
                            # read PSUM); host divides. One 2*OW-wide stage
                            # per half-unit.
                            if STAGE_ACT:
                                nc.scalar.activation(
                                    o_grp[:, ob0:ob0 + 2 * OW],
                                    acc[:, :],
                                    mybir.ActivationFunctionType.Copy,
                                    scale=1.0,
                                )
                            else:
                                nc.vector.tensor_copy(
                                    o_grp[:, ob0:ob0 + 2 * OW],
                                    acc[:, :],
                                )
                        else:
                            for cc in (c - 1, c):
                                a0 = (cc % 2) * VW
                                r = rrp.tile([128, 1], f32, tag="r",
                                             name=f"r{h}g{g}c{cc}")
                                nc.vector.reciprocal(
                                    r[:, :], acc[:, a0 + D:a0 + D + 1])
                                nc.vector.tensor_scalar_mul(
                                    o_grp[:, cc * OW:(cc + 1) * OW],
                                    acc[:, a0:a0 + D], r[:, :],
                                )
                        if split_store and c == 1:
                            store_eng.dma_start(
                                out=out_d[h, :,
                                          4 * g * OW:(4 * g + 2) * OW],
                                in_=o_grp[:, 0:2 * OW])
                        elif split_store and c == 3:
                            store_eng.dma_start(
                                out=out_d[h, :,
                                          (4 * g + 2) * OW:(4 * g + 4) * OW],
                                in_=o_grp[:, 2 * OW:4 * OW])
                        elif not split_store and c == 3:
                            store_eng.dma_start(
                                out=out_d[h, :,
                                          4 * g * OW:(4 * g + 4) * OW],
                                in_=o_grp[:, :])
                    yield 0, fin

            def make_unit(h, g, bufs, split_all=False):
                # Chunk layout: non-diag js in twos (512 each, bank aligned),
                # then the diag chunk packed 512+384 | 256+128 into 2.5 banks.
                # chunks: list of (pt_col0, [(j, qlo, off_in_chunk, w)], width)
                chunks = []
                col = 0
                nd = 4 * g  # non-diagonal k-blocks
                for i0 in range(0, nd, 2):
                    entries = [
                        (j, GW * g, (j - i0) * GW, GW)
                        for j in range(i0, min(i0 + 2, nd))
                    ]
                    cw = len(entries) * GW
                    chunks.append((col, entries, cw))
                    col += cw
                d0 = 4 * g
                if split_all:
                    # per-j chunks (used for the final unit so its PV can
                    # begin before the whole diagonal chunk is exp'd)
                    for i, w in enumerate((512, 384, 256, 128)):
                        chunks.append(
                            (col, [(d0 + i, QB * (d0 + i), 0, w)], w)
                        )
                        col += w
                else:
                    chunks.append((col, [
                        (d0, QB * d0, 0, 512),
                        (d0 + 1, QB * (d0 + 1), 512, 384),
                    ], 896))
                    col += 896
                    chunks.append((col, [
                        (d0 + 2, QB * (d0 + 2), 0, 256),
                        (d0 + 3, QB * (d0 + 3), 256, 128),
                    ], 384))
                    col += 384
                offs = {}
                for col0, entries, _ in chunks:
                    for (j, qlo, off, _w) in entries:
                        offs[j] = (qlo, col0 + off)
                pt_sb = ptp.tile(
                    [128, 12 * GW + 1280], bf16, tag="pt", name=f"pt{h}g{g}"
                )
                return (h, g, bufs, pt_sb, offs, chunks)

            # Global clock-based pacing: emit exp chunks on each exp-engine's
            # schedule, fill PE's spare time from a queue of pending PV work.
            # Clocks in PE cycles @2.4GHz; ACT cycles count double, DVE 2.5x,
            # GPS 3.33x. chunk_end tracks modeled exp completions so chunk
            # k's matmuls are delayed until chunk k-3's PSUM slot frees
            # (3-slot st pool) with PV filler emitted in the meantime.
            pe_clock = 0.0
            eng_clock = {"act": 0.0, "dve": 0.0, "gps": 0.0}
            SEMC = 240.0  # ~100ns handoff latency in PE cycles
            chunk_end = []
            pvq = []  # list of (unit_idx, pe_cost, closure), FIFO
            qi = 0

            def drain_pv(upto_unit=None, clock_limit=None, nsteps=None):
                nonlocal qi, pe_clock
                done = 0
                while qi < len(pvq):
                    uidx, cost, fn = pvq[qi]
                    if upto_unit is not None and uidx > upto_unit:
                        break
                    if clock_limit is not None and pe_clock >= clock_limit:
                        break
                    if nsteps is not None and done >= nsteps:
                        break
                    fn()
                    pe_clock += cost
                    qi += 1
                    done += 1

            head_bufs = [None] * HPC
            head_bufs[0] = load_head(0)
            uidx = 0
            for h in range(HPC):
                if h + 1 < HPC:
                    head_bufs[h + 1] = load_head(h + 1)
                if h == HPC - 1:
                    gs = [int(x) for x in _os.environ.get(
                        "LAST_ORDER", "2,3,1,0").split(",")]
                elif h == 0:
                    gs = H0_ORDER
                else:
                    gs = G_ORDER
                for g in gs:
                    # pt pool has PT_BUFS slots: before unit uidx's first exp
                    # can run, unit uidx-PT_BUFS's PV (the slot's previous
                    # holder's reader) must be fully emitted on PE's stream.
                    drain_pv(upto_unit=uidx - PT_BUFS)
                    last = uidx == NG * HPC - 1
                    bufs_u = head_bufs[h]
                    if h == 0 and g == 0:
                        bufs_u = (p0_qt, p0_kt) + tuple(bufs_u[2:])
                    u = make_unit(h, g, bufs_u, split_all=last)
                    if last:
                        # endgame: per-j chunks; leftover PV of previous units
                        # drains under the first exp; each own PV chain goes
                        # right after the per-j exp it depends on
                        own = list(pv_steps(u, split_store=True))
                        oi = 0
                        for ci, (mms, mmcost, ex, eng, ecost) in enumerate(
                            s_chunks(u)
                        ):
                            if ci == 0:
                                drain_pv()
                            for mm in mms:
                                mm()
                            ex()
                            for _ in range(ci + 2):  # ~c+1 matmuls + fin
                                if oi < len(own):
                                    own[oi][1]()
                                    oi += 1
                        while oi < len(own):
                            own[oi][1]()
                            oi += 1
                        continue
                    for mms, mmcost, ex, eng, ecost in s_chunks(u):
                        # give PE filler work until this chunk's exp engine
                        # and its PSUM slot are about to be available
                        target = eng_clock[eng]
                        if len(chunk_end) >= ST_BUFS:
                            target = max(target, chunk_end[-ST_BUFS])
                        drain_pv(clock_limit=target - mmcost)
                        for mm in mms:
                            mm()
                        pe_clock += mmcost
                        if eng != "act" and EX_DELAY:
                            drain_pv(nsteps=EX_DELAY)
                        ex()
                        e_end = max(eng_clock[eng], pe_clock + SEMC) + ecost
                        eng_clock[eng] = e_end
                        chunk_end.append(e_end)
                    pvq.extend(
                        (uidx, cost, fn) for cost, fn in pv_steps(u)
                    )
                    uidx += 1
            drain_pv()

    nc.compile()
    return nc


_F8 = ml_dtypes.float8_e4m3


def _e4m3(x):
    return np.clip(x, -240.0, 240.0).astype(_F8)


def _prep_core(q, k, v):
    """q,k,v: [HPC, S, D] f32 for one core -> device input dict."""
    qtf = np.ascontiguousarray(q.transpose(0, 2, 1))  # [HPC, d, S] f32
    ktf = np.ascontiguousarray(k.transpose(0, 2, 1))
    qt = qtf.astype(_BF16)
    kt = ktf.astype(_BF16)
    va = np.empty((HPC, S, VW), dtype=np.float32)
    va[:, :, :D] = v
    va[:, :, D] = 1.0
    # [HPC, S, VW] -> [HPC, 128, NJ*VW]  with [p, j*VW+c] = va[j*128+p, c]
    va = np.ascontiguousarray(
        va.reshape(HPC, NJ, QB, VW).transpose(0, 2, 1, 3)
    ).reshape(HPC, QB, NJ * VW).astype(_BF16)
    m = {"qt": qt, "kt": kt, "va": va}
    fp8_js, fp8_gs = _CACHE.get("fp8", ([], []))
    if fp8_js:
        # K at double-fp8 (hi + lo/16), Q one-sided e4m3 (+ /16 copy for the
        # DoubleRow second slot)
        kcols = np.concatenate(
            [ktf[:, :, j * QB:(j + 1) * QB] for j in fp8_js], axis=2
        )
        k_hi = _e4m3(kcols)
        k_lo = _e4m3((kcols - k_hi.astype(np.float32)) * 16.0)
        m["khl"] = np.stack([k_hi, k_lo], axis=2)  # [HPC, 128, 2, NJ8*QB]
        qcols = np.concatenate(
            [qtf[:, :, g * GW:(g + 1) * GW] for g in fp8_gs], axis=2
        )
        q8 = _e4m3(qcols)
        q8s = (q8.astype(np.float32) / 16.0).astype(_F8)
        m["q8"] = np.stack([q8, q8s], axis=2)      # [HPC, 128, 2, NG8*GW]
    return m


def _run(query, key, value, trace=False):
    from concourse import bass_utils

    if "nc" not in _CACHE:
        _CACHE["nc"] = _build()
    nc = _CACHE["nc"]

    q = np.asarray(query, dtype=np.float32).reshape(B * H, S, D)
    k = np.asarray(key, dtype=np.float32).reshape(B * H, S, D)
    v = np.asarray(value, dtype=np.float32).reshape(B * H, S, D)
    if _CACHE["mask_mm"]:
        negi = (-60000.0 * np.eye(128, dtype=np.float32)).astype(_BF16)
    else:
        negi = np.triu(np.ones((128, 128), dtype=np.float32)).astype(_BF16)
    low1 = np.tril(np.ones((128, 128), dtype=np.float32), -1).astype(_BF16)

    in_maps = []
    for c in range(N_CORES):
        sl = slice(c * HPC, (c + 1) * HPC)
        m = _prep_core(q[sl], k[sl], v[sl])
        # packed startup tile: [negI | lower1 | kt0 0:512 | qt0 0:512]
        m["p0"] = np.concatenate(
            [negi, low1, m["kt"][0][:, :512], m["qt"][0][:, :512]], axis=1
        )
        in_maps.append(m)

    res = bass_utils.run_bass_kernel_spmd(
        nc, in_maps, core_ids=list(range(N_CORES)), trace=trace
    )
    outs = [res.results[c]["out"] for c in range(N_CORES)]
    full = np.concatenate(outs, axis=0).astype(np.float32)
    # device layout is partition-major: out[h, p, j*OW + d] = row q = j*128+p
    OW = full.shape[-1] // NJ
    full = full.reshape(B * H, QB, NJ, OW).transpose(0, 2, 1, 3)
    full = np.ascontiguousarray(full).reshape(B * H, S, OW)
    if _CACHE.get("norm_host"):
        full = full[..., :D] / full[..., D:D + 1]
    full = full.reshape(B, H, S, D)
    return full, res


def kernel(query, key, value, mask=None):
    """Full inputs in, full output out. `mask` is the causal mask from
    setup_inputs (strictly-upper-triangular True = disallowed); causality is
    implemented structurally so the tensor itself is not consumed."""
    out, _ = _run(query, key, value, trace=False)
    return out



# revision 15
# speedup vs baseline: 1.2947x; 1.0226x over previous
"""Causal multi-head attention on 8 TRN2 NeuronCores.

Problem: B=4, H=16, S=2048, D=128 fp32 causal attention.
Sharding: batch*heads (64) split 8-per-core across the 8 cores; each core
computes its heads fully independently (no collectives).

Per-core kernel strategy (f32 accumulation):
  - scores computed TRANSPOSED: S^T[k,q] = K_j @ Q^T per (k-block j of 128,
    q-group g of 512), causal blocks only, into 2-bank PSUM chunks (<=1024)
    with a 3-deep chunk pipeline
  - ALL non-diagonal score blocks of groups g2/g3 run as fp8e4 DoubleRow
    matmuls at 2x PE rate (cost-model 0.5 cycles/row): the stationary K side
    carries hi + lo/16 double-fp8 (~8 mantissa bits), the moving Q side is
    one-sided e4m3 with a /16 second slot. Host pre-quantizes Q/K; measured
    output rel err ~1.0% vs the 2e-2 gate. fp8 on g0/g1/diagonal blocks is
    deliberately avoided: early rows have few keys and large output norms,
    so they dominate the error weighting.
  - exp mostly on ScalarE (PSUM -> SBUF bf16, 1/sqrt(D) folded into the
    activation scale); four late chunks per head offloaded to VectorE via an
    exp2 bit-trick (one tensor_scalar: bf16 bits = rint(x*log2e*128 +
    127*128 - 7.5), f32->int16 convert aliased over the bf16 tile)
  - diagonal 128x128 blocks masked with a constant triangular tile on DVE
  - PV: out[q,:] = P^T_slice.T @ [V_j | ones]; the appended ones column
    yields the softmax denominator in the same accumulation. Output lands
    directly in [q, d] layout.
  - normalize with VectorE reciprocal + per-partition tensor_scalar mult
  - packed single-DMA startup tile (mask consts + first K/Q block); per-head
    unit order g0,g3,g1,g2 balances exp-column supply against PV drains;
    per-engine pacing clocks keep the PE fed with PV filler work
"""

import sys

import numpy as np
import ml_dtypes

for _p in ("/opt/trn_rl_repo", "/root/.axon_site/_ro/trn_rl_repo"):
    try:
        import concourse  # noqa: F401
        break
    except ImportError:
        if _p not in sys.path:
            sys.path.append(_p)

B, H, S, D = 4, 16, 2048, 128
N_CORES = 8
HPC = (B * H) // N_CORES  # heads per core = 8
QB = 128                  # q/k block
GW = 512                  # q group width
NG = S // GW              # 4 groups per head
NJ = S // QB              # 16 k blocks
VW = D + 1                # V augmented with ones column = 129
SCALE = 1.0 / float(np.sqrt(D))

_BF16 = ml_dtypes.bfloat16

_CACHE = {}


def _build():
    import concourse.bass as bass  # noqa: F401
    import concourse.mybir as mybir
    from concourse import bacc
    from concourse.tile import TileContext

    f32 = mybir.dt.float32
    bf16 = mybir.dt.bfloat16
    i16 = mybir.dt.int16
    EXP = mybir.ActivationFunctionType.Exp
    # chunks routed to the bit-trick exp on VectorE / GpSimd: (g, chunk_idx).
    # Late chunks only — their PV consumption comes last in each q-chain, so
    # the slower engines' latency hides behind ScalarE's pipeline.
    import os as _os
    _dve = _os.environ.get("DVE_SET",
                           "(3,0);(3,1);(3,2);(3,3);(3,4);(3,5)")
    _gps = _os.environ.get("GPS_SET", "")

    def _parse(s):
        out = set()
        for part in s.split(";"):
            part = part.strip().strip("()")
            if part:
                a, b = part.split(",")
                out.add((int(a), int(b)))
        return out

    DVE_CHUNKS = _parse(_dve)
    GPS_CHUNKS = _parse(_gps)
    # per-chunk column split: DVE takes the LAST `SPLIT_COLS` columns of each
    # non-diag ACT chunk (short DVE ops so diag masks aren't queued behind
    # long ones), ScalarE the rest
    SPLIT_COLS = int(_os.environ.get("SPLIT_COLS", "54"))
    NORM_GPS = _os.environ.get("NORM_GPS", "0") == "1"
    # PV steps drained between an offloaded chunk's matmuls and its exp
    # emission: their recip/norm ops enter the DVE FIFO before the exp, so
    # the exp's wait-for-matmuls doesn't head-of-line-block them
    EX_DELAY = int(_os.environ.get("EX_DELAY", "0"))
    # NORM_HOST: the device ships the unnormalized numerator plus the
    # denominator column (VW wide), and the host performs the final divide.
    # (GPSIMD cannot read PSUM, so the acc->SBUF stage stays on DVE either
    # way; host-normalize still drops the reciprocal+multiply.)
    NORM_HOST = _os.environ.get("NORM_HOST", "1") == "1"
    # MASK_GPS: diag masks (SBUF-only tensor_mul) run on the idle GpSimd
    MASK_GPS = _os.environ.get("MASK_GPS", "1") == "1"
    # STAGE_ACT: the acc->SBUF stage runs on ScalarE (activation Copy)
    # instead of DVE; STAGE_BF16: stage+store in bf16 (halves store traffic)
    STAGE_ACT = _os.environ.get("STAGE_ACT", "0") == "1"
    STAGE_BF16 = _os.environ.get("STAGE_BF16", "1") == "1"
    # per-head unit order (steady-state heads): balances exp-column supply
    # against PV-drain bursts at head boundaries
    G_ORDER = [int(x) for x in
               _os.environ.get("G_ORDER", "0,3,1,2").split(",")]
    # head 0 ramps with incremental data needs (g1 needs far less than g3),
    # so its unit order favors load streaming over engine balance
    H0_ORDER = [int(x) for x in
                _os.environ.get("H0_ORDER", "0,1,2,3").split(",")]
    ACT_COST_SCALE = float(_os.environ.get("ACT_COST_SCALE", "0.85"))
    ST_BUFS = int(_os.environ.get("ST_BUFS", "3"))
    ACC_BUFS = int(_os.environ.get("ACC_BUFS", "2"))
    PT_BUFS = int(_os.environ.get("PT_BUFS", "3"))
    MASK_MM = _os.environ.get("MASK_MM", "1") == "1"
    # non-diag chunks whose score matmuls run as fp8e4 DoubleRow (2x PE rate;
    # K carried at hi+lo/16 double-fp8 precision, Q one-sided e4m3).
    # FP8_DIAG_GS: groups whose DIAGONAL blocks also run fp8 (g0's diagonal
    # carries the highest softmax output weight — early rows have few keys
    # and large output norms — so it stays bf16).
    FP8_CHUNKS = _parse(_os.environ.get(
        "FP8_SET", "(1,0);(1,1);"
                   "(2,0);(2,1);(2,2);(2,3);"
                   "(3,0);(3,1);(3,2);(3,3);(3,4);(3,5)"))
    FP8_DIAG_GS = {int(x) for x in
                   _os.environ.get("FP8_DIAG_GS", "1,2,3").split(",")
                   if x.strip()}
    FP8_JS = sorted(
        {j for (g, ci) in FP8_CHUNKS for j in (2 * ci, 2 * ci + 1)
         if j < 4 * g}
        | {4 * g + i for g in FP8_DIAG_GS for i in range(4)}
    )
    FP8_GS = sorted({g for (g, ci) in FP8_CHUNKS} | FP8_DIAG_GS)
    NJ8 = len(FP8_JS)
    J8_COL = {j: i * QB for i, j in enumerate(FP8_JS)}   # khl col offset
    G8_COL = {g: i * GW for i, g in enumerate(FP8_GS)}   # q8 col offset
    # bf16 residual needs: which qt group windows / kt j-blocks still load
    BF16_GS = sorted(
        {g for g in range(NG) for ci in range(2 * g)
         if (g, ci) not in FP8_CHUNKS}
        | {g for g in range(NG) if g not in FP8_DIAG_GS}
    )
    BF16_JS = sorted(
        {j for g in range(NG) for ci in range(2 * g)
         if (g, ci) not in FP8_CHUNKS for j in (2 * ci, 2 * ci + 1)}
        | {4 * g + i for g in range(NG) if g not in FP8_DIAG_GS
           for i in range(4)}
    )
    assert 0 not in FP8_DIAG_GS, "g0 diag must stay bf16 (p0 startup path)"
    _CACHE["mask_mm"] = MASK_MM
    KT_COL = {j: i * QB for i, j in enumerate(BF16_JS)}
    QT_COL = {g: i * GW for i, g in enumerate(BF16_GS)}
    C1E = float(np.log2(np.e) * 128.0 * SCALE)
    C2E = 127.0 * 128.0 - 7.5

    nc = bacc.Bacc("TRN2", target_bir_lowering=False, num_devices=N_CORES)

    qt_d = nc.dram_tensor("qt", [HPC, 128, S], bf16, kind="ExternalInput").ap()
    kt_d = nc.dram_tensor("kt", [HPC, 128, S], bf16, kind="ExternalInput").ap()
    va_d = nc.dram_tensor("va", [HPC, 128, NJ * VW], bf16, kind="ExternalInput").ap()
    # packed startup tile: [negI | lower1 | kt0 0:512 | qt0 0:512] —
    # mask-matmul constants plus everything head-0's g0 diag chunk needs,
    # fetched in a single DMA (per-DMA fixed costs dominate the startup
    # critical path)
    p0_d = nc.dram_tensor("p0", [128, 2 * QB + 2 * GW], bf16,
                          kind="ExternalInput").ap()
    f8e4 = mybir.dt.float8e4
    DRPM = mybir.MatmulPerfMode.DoubleRow
    if NJ8:
        khl_d = nc.dram_tensor("khl", [HPC, 128, 2, NJ8 * QB], f8e4,
                               kind="ExternalInput").ap()
        q8_d = nc.dram_tensor("q8", [HPC, 128, 2, len(FP8_GS) * GW], f8e4,
                              kind="ExternalInput").ap()
    _CACHE["fp8"] = (FP8_JS, FP8_GS)
    _CACHE["norm_host"] = NORM_HOST
    OW = VW if NORM_HOST else D   # output row width
    out_dt = bf16 if STAGE_BF16 else f32
    _CACHE["out_bf16"] = STAGE_BF16
    # partition-major out layout: out[h, p, j*OW + d] holds row q = j*128 + p.
    # Stores become fully contiguous per partition (4*OW*2B = 1032B runs at
    # full DMA bus rate vs 258B rows at half rate); host un-shuffles.
    out_d = nc.dram_tensor("out", [HPC, 128, NJ * OW], out_dt,
                           kind="ExternalOutput").ap()

    with TileContext(nc) as tc:
        with (
            tc.tile_pool(name="consts", bufs=1) as consts,
            tc.tile_pool(name="io", bufs=3) as io,
            tc.tile_pool(name="pt", bufs=PT_BUFS) as ptp,
            tc.tile_pool(name="ob", bufs=4) as obp,
            tc.tile_pool(name="rr", bufs=4) as rrp,
            tc.tile_pool(name="st", bufs=ST_BUFS, space="PSUM") as stp,
            tc.tile_pool(name="acc", bufs=ACC_BUFS, space="PSUM") as accp,
        ):
            # packed startup tile [negI | lower1 | kt0 0:512 | qt0 0:512]:
            # one DMA with one completion-sem covers everything unit (h0, g0)
            # consumes. negI/lower1 implement causal masking INSIDE the score
            # accumulation: st[k,q] += -60000*[k>q] via one extra 128-col
            # matmul per diag block, so exp yields exact zeros above the
            # diagonal and no post-exp mask op exists on any engine.
            p0_sb = consts.tile([128, 2 * QB + 2 * GW], bf16, name="p0_sb")
            nc.sync.dma_start(out=p0_sb[:, :], in_=p0_d[:, :])
            negi_sb = p0_sb[:, 0:QB]
            low1_sb = p0_sb[:, QB:2 * QB]
            p0_kt = p0_sb[:, 2 * QB:2 * QB + GW]
            p0_qt = p0_sb[:, 2 * QB + GW:2 * QB + 2 * GW]
            # HAM warm-up: PE idles ~2us at start waiting for the first DMA
            # anyway; dummy matmuls on memset SBUF keep the PE activity
            # monitor busy so real work starts fast (real-HW; near-neutral in
            # sim). PSUM target is overwritten by start=True.
            warm_in = consts.tile([128, VW], bf16, name="warm_in")
            nc.vector.memset(warm_in[:, :], 0.0)
            # hoist the ACT table load (real-HW ~2.7us incl drain) into the
            # startup DMA window via a dummy 1-col activation
            warm_ex = consts.tile([128, 1], bf16, name="warm_ex")
            nc.scalar.activation(warm_ex[:, :], warm_in[:, 0:1], EXP,
                                 scale=SCALE)
            warm_acc = accp.tile([128, VW], f32, tag="acc", name="warm_acc")
            for _ in range(16):
                nc.tensor.matmul(
                    warm_acc[:, :], lhsT=warm_in[:, 0:QB],
                    rhs=warm_in[:, 0:VW], start=True, stop=True,
                )

            def load_head(h):
                """bf16 qt/kt tiles hold only the group-windows / j-blocks
                still computed in bf16 (KT_COL/QT_COL give their offsets);
                fp8 khl/q8 carry everything else. All loads issue on SP
                (HWDGE) so the Pool engine stays free for the diag masks."""
                nq = max(len(BF16_GS), 1)
                nk = max(len(BF16_JS), 1)
                qt_sb = io.tile([128, nq * GW], bf16, tag="qt", name=f"qt{h}")
                kt_sb = io.tile([128, nk * QB], bf16, tag="kt", name=f"kt{h}")
                va_sb = io.tile([128, NJ * VW], bf16, tag="va", name=f"va{h}")
                if NJ8:
                    khl_sb = io.tile([128, 2, NJ8 * QB], f8e4, tag="khl",
                                     name=f"khl{h}")
                    q8_sb = io.tile([128, 2, len(FP8_GS) * GW], f8e4,
                                    tag="q8", name=f"q8{h}")
                else:
                    khl_sb = q8_sb = None
                m = (NJ * VW) // 2
                skip_p0 = h == 0
                # p0 fully covers head 0's bf16 needs when the bf16 residue
                # is exactly g0's diagonal (all-fp8 nd + fp8 diag g1..g3)
                p0_covers = (skip_p0 and BF16_JS == [0, 1, 2, 3]
                             and BF16_GS == [0])

                def kt_runs():
                    runs = []
                    for j in BF16_JS:
                        if runs and runs[-1][1] == j:
                            runs[-1][1] = j + 1
                        else:
                            runs.append([j, j + 1])
                    return runs

                def kt_piece(j0, j1):
                    if j0 >= j1:
                        return
                    nc.sync.dma_start(
                        out=kt_sb[:, KT_COL[j0]:KT_COL[j0] + (j1 - j0) * QB],
                        in_=kt_d[h, :, j0 * QB:j1 * QB])

                def qt_piece(g):
                    nc.sync.dma_start(
                        out=qt_sb[:, QT_COL[g]:QT_COL[g] + GW],
                        in_=qt_d[h, :, g * GW:(g + 1) * GW])

                def q8_piece(g):
                    c = G8_COL[g]
                    nc.sync.dma_start(out=q8_sb[:, :, c:c + GW],
                                      in_=q8_d[h, :, :, c:c + GW])

                if not p0_covers:
                    for j0, j1 in kt_runs():
                        kt_piece(j0, j1)
                    for g in BF16_GS:
                        if skip_p0 and g == 0:
                            continue
                        qt_piece(g)
                # stream in first-use order of the head's unit order:
                # va first half feeds g0's PV early; q8 pieces follow the
                # group order; khl halves bracket them
                order = H0_ORDER if h == 0 else G_ORDER
                nc.sync.dma_start(out=va_sb[:, 0:m], in_=va_d[h, :, 0:m])
                if NJ8:
                    half = (NJ8 * QB) // 2
                    nc.sync.dma_start(out=khl_sb[:, :, 0:half],
                                      in_=khl_d[h, :, :, 0:half])
                    gs8 = [g for g in order if g in FP8_GS]
                    if gs8:
                        q8_piece(gs8[0])
                    nc.sync.dma_start(out=khl_sb[:, :, half:],
                                      in_=khl_d[h, :, :, half:])
                    for g in gs8[1:2]:
                        q8_piece(g)
                    nc.sync.dma_start(out=va_sb[:, m:], in_=va_d[h, :, m:])
                    for g in gs8[2:]:
                        q8_piece(g)
                else:
                    nc.sync.dma_start(out=va_sb[:, m:], in_=va_d[h, :, m:])
                return qt_sb, kt_sb, va_sb, khl_sb, q8_sb

            def s_chunks(u):
                """Per chunk of unit u: (mm_closures_with_cost, exp_closure,
                act_cost). S^T matmuls land in bank-aligned PSUM chunks
                (<=3 banks), one exp per chunk, diag masks after the exp."""
                h, g, bufs, pt_sb, offs, chunks = u
                qt_sb, kt_sb = bufs[0], bufs[1]
                khl_sb, q8_sb = bufs[3], bufs[4]
                qhi = GW * (g + 1)
                for ci, (col0, entries, cw) in enumerate(chunks):
                    st = stp.tile([128, 1024], f32, tag="st",
                                  name=f"st{h}g{g}c{ci}")
                    mms = []
                    mmcost = 0
                    fp8c = (g, ci) in FP8_CHUNKS and all(
                        j < 4 * g for (j, _q, _o, _w) in entries
                    )
                    for (j, qlo, off, w) in entries:
                        if fp8c or (j >= 4 * g and g in FP8_DIAG_GS):
                            diag_mask = j >= 4 * g and MASK_MM

                            def mm(j=j, qlo=qlo, off=off, w=w, st=st, g=g,
                                   diag_mask=diag_mask):
                                c8 = G8_COL[g] + (qlo - GW * g)
                                nc.tensor.matmul(
                                    st[:, off:off + w],
                                    lhsT=khl_sb[
                                        :, :, J8_COL[j]:J8_COL[j] + QB
                                    ],
                                    rhs=q8_sb[:, :, c8:c8 + w],
                                    start=True, stop=not diag_mask,
                                    perf_mode=DRPM,
                                )
                                if diag_mask:
                                    # causal mask inside the accumulation:
                                    # st[k,q] += -60000*[k>q] on the leading
                                    # 128-col diagonal square of the entry
                                    nc.tensor.matmul(
                                        st[:, off:off + QB],
                                        lhsT=negi_sb[:, :],
                                        rhs=low1_sb[:, :],
                                        start=False, stop=True,
                                        skip_group_check=True,
                                    )
                            mms.append(mm)
                            mmcost += w // 2 + 8
                            if diag_mask:
                                mmcost += QB + 8
                        elif j >= 4 * g and MASK_MM:
                            # diag block: accumulate -60000 above the diagonal
                            # in the same PSUM group (masking via the PE)
                            def mm(j=j, qlo=qlo, off=off, w=w, st=st, g=g):
                                cq = QT_COL[g] + (qlo - GW * g)
                                nc.tensor.matmul(
                                    st[:, off:off + w],
                                    lhsT=kt_sb[:, KT_COL[j]:KT_COL[j] + QB],
                                    rhs=qt_sb[:, cq:cq + w],
                                    start=True, stop=False,
                                )
                                nc.tensor.matmul(
                                    st[:, off:off + QB],
                                    lhsT=negi_sb[:, :],
                                    rhs=low1_sb[:, :],
                                    start=False, stop=True,
                                    skip_group_check=True,
                                )
                            mms.append(mm)
                            mmcost += w + QB + 16
                        else:
                            def mm(j=j, qlo=qlo, off=off, w=w, st=st, g=g):
                                cq = QT_COL[g] + (qlo - GW * g)
                                nc.tensor.matmul(
                                    st[:, off:off + w],
                                    lhsT=kt_sb[:, KT_COL[j]:KT_COL[j] + QB],
                                    rhs=qt_sb[:, cq:cq + w],
                                    start=True, stop=True,
                                )
                            mms.append(mm)
                            mmcost += w + 8

                    # offload a slice of the exp work to the idle VectorE /
                    # GpSimd engines via the exp2 bit-trick: bf16 bits =
                    # rint(x*log2e*128 + C2E) (one tensor_scalar, f32->int16
                    # convert aliased over the bf16 tile). ~1.3% element
                    # error, bias-centered so the softmax output error stays
                    # small. (the split-all final unit keeps everything on
                    # ScalarE)
                    final = g == 0 and len(chunks) > 1
                    nd_chunk = all(j < 4 * g for (j, _q, _o, _w) in entries)
                    eng = "act"
                    if not final and (nd_chunk or not MASK_MM):
                        # (with MASK_MM the bit-trick's f32->int16 convert
                        # would wrap on the -60000 masked scores, so diag
                        # chunks are only offloadable on the DVE-mask path)
                        if (g, ci) in DVE_CHUNKS:
                            eng = "dve"
                        elif (g, ci) in GPS_CHUNKS:
                            eng = "gps"

                    sl = SPLIT_COLS if (eng == "act" and nd_chunk and not final
                                        and SPLIT_COLS < cw) else 0

                    def ex(col0=col0, cw=cw, st=st, entries=entries, eng=eng,
                           sl=sl):
                        if eng == "act":
                            nc.scalar.activation(
                                pt_sb[:, col0:col0 + cw - sl],
                                st[:, 0:cw - sl], EXP, scale=SCALE,
                            )
                            if sl:
                                nc.vector.tensor_scalar(
                                    pt_sb[
                                        :, col0 + cw - sl:col0 + cw
                                    ].bitcast(i16),
                                    st[:, cw - sl:cw], C1E, C2E,
                                    mybir.AluOpType.mult, mybir.AluOpType.add,
                                )
                        else:
                            e = nc.vector if eng == "dve" else nc.gpsimd
                            e.tensor_scalar(
                                pt_sb[:, col0:col0 + cw].bitcast(i16),
                                st[:, 0:cw], C1E, C2E,
                                mybir.AluOpType.mult, mybir.AluOpType.add,
                            )
                        if not MASK_MM:
                            # negi_sb slot holds the inclusive upper-tri mask
                            # in this mode (host-selected)
                            me = nc.gpsimd if MASK_GPS else nc.vector
                            for (j, qlo, off, w) in entries:
                                if j >= 4 * g:  # diag: zero where k > q
                                    me.tensor_mul(
                                        pt_sb[:, col0 + off:col0 + off + QB],
                                        pt_sb[:, col0 + off:col0 + off + QB],
                                        negi_sb[:, :],
                                    )
                    if eng == "act":
                        ecost = int(2 * (cw - sl + 222) * ACT_COST_SCALE)
                    elif eng == "dve":
                        ecost = int(2.5 * cw) + 300
                    else:
                        ecost = int(3.33 * cw) + 700
                    yield mms, mmcost, ex, eng, ecost

            def pv_steps(u, split_store=False, store_eng=None):
                """(pe_cost, closure) steps: PV accumulation matmuls +
                stage + store for unit u. acc tiles hold HALF a unit
                (2 q-blocks, exactly 1 PSUM bank) so 3 score chunks + 2 accs
                fit the 8 banks. Staging happens per half (one 2*OW-col op),
                the store once per unit (split_store also stores the first
                half early for the endgame tail)."""
                h, g, bufs, pt_sb, offs, _chunks = u
                va_sb = bufs[2]
                if store_eng is None:
                    store_eng = nc.sync
                o_grp = obp.tile([128, 4 * OW], out_dt, tag="obg",
                                 name=f"og{h}g{g}")
                acc = None
                for c in range(4):
                    Q = 4 * g + c
                    qlo_c = GW * g + QB * c
                    if c % 2 == 0:
                        acc = accp.tile([128, 2 * VW], f32, tag="acc",
                                        name=f"acc{h}g{g}c{c}")
                    a0 = (c % 2) * VW
                    for j in range(Q + 1):
                        qlo_j, col_j = offs[j]
                        off = col_j + (qlo_c - qlo_j)

                        def step(j=j, Q=Q, off=off, acc=acc, a0=a0):
                            nc.tensor.matmul(
                                acc[:, a0:a0 + VW],
                                lhsT=pt_sb[:, off:off + QB],
                                rhs=va_sb[:, j * VW:(j + 1) * VW],
                                start=(j == 0), stop=(j == Q),
                            )
                        yield 300, step

                    if c % 2 == 0:
                        continue

                    def fin(c=c, acc=acc, o_grp=o_grp):
                        ob0 = (c - 1) * OW
                        if NORM_HOST:
                            # ship numerator+denominator (only ACT/DVE can
                            # read PSUM); host divides. One 2*OW-wide stage
                            # per half-unit.
                            if STAGE_ACT:
                                nc.scalar.activation(
                                    o_grp[:, ob0:ob0 + 2 * OW],
                                    acc[:, :],
                                    mybir.ActivationFunctionType.Copy,
                                    scale=1.0,
                                )
                            else:
                                nc.vector.tensor_copy(
                                    o_grp[:, ob0:ob0 + 2 * OW],
                                    acc[:, :],
                                )
                        else:
                            for cc in (c - 1, c):
                                a0 = (cc % 2) * VW
                                r = rrp.tile([128, 1], f32, tag="r",
                                             name=f"r{h}g{g}c{cc}")
                                nc.vector.reciprocal(
                                    r[:, :], acc[:, a0 + D:a0 + D + 1])
                                nc.vector.tensor_scalar_mul(
                                    o_grp[:, cc * OW:(cc + 1) * OW],
                                    acc[:, a0:a0 + D], r[:, :],
                                )
                        if split_store and c == 1:
                            store_eng.dma_start(
                                out=out_d[h, :,
                                          4 * g * OW:(4 * g + 2) * OW],
                                in_=o_grp[:, 0:2 * OW])
                        elif split_store and c == 3:
                            store_eng.dma_start(
                                out=out_d[h, :,
                                          (4 * g + 2) * OW:(4 * g + 4) * OW],
                                in_=o_grp[:, 2 * OW:4 * OW])
                        elif not split_store and c == 3:
                            store_eng.dma_start(
                                out=out_d[h, :,
                                          4 * g * OW:(4 * g + 4) * OW],
                                in_=o_grp[:, :])
                    yield 0, fin

            def make_unit(h, g, bufs, split_all=False):
                # Chunk layout: non-diag js in twos (512 each, bank aligned),
                # then the diag chunk packed 512+384 | 256+128 into 2.5 banks.
                # chunks: list of (pt_col0, [(j, qlo, off_in_chunk, w)], width)
                chunks = []
                col = 0
                nd = 4 * g  # non-diagonal k-blocks
                for i0 in range(0, nd, 2):
                    entries = [
                        (j, GW * g, (j - i0) * GW, GW)
                        for j in range(i0, min(i0 + 2, nd))
                    ]
                    cw = len(entries) * GW
                    chunks.append((col, entries, cw))
                    col += cw
                d0 = 4 * g
                if split_all:
                    # per-j chunks (used for the final unit so its PV can
                    # begin before the whole diagonal chunk is exp'd)
                    for i, w in enumerate((512, 384, 256, 128)):
                        chunks.append(
                            (col, [(d0 + i, QB * (d0 + i), 0, w)], w)
                        )
                        col += w
                else:
                    chunks.append((col, [
                        (d0, QB * d0, 0, 512),
                        (d0 + 1, QB * (d0 + 1), 512, 384),
                    ], 896))
                    col += 896
                    chunks.append((col, [
                        (d0 + 2, QB * (d0 + 2), 0, 256),
                        (d0 + 3, QB * (d0 + 3), 256, 128),
                    ], 384))
                    col += 384
                offs = {}
                for col0, entries, _ in chunks:
                    for (j, qlo, off, _w) in entries:
                        offs[j] = (qlo, col0 + off)
                pt_sb = ptp.tile(
                    [128, 12 * GW + 1280], bf16, tag="pt", name=f"pt{h}g{g}"
                )
                return (h, g, bufs, pt_sb, offs, chunks)

            # Global clock-based pacing: emit exp chunks on each exp-engine's
            # schedule, fill PE's spare time from a queue of pending PV work.
            # Clocks in PE cycles @2.4GHz; ACT cycles count double, DVE 2.5x,
            # GPS 3.33x. chunk_end tracks modeled exp completions so chunk
            # k's matmuls are delayed until chunk k-3's PSUM slot frees
            # (3-slot st pool) with PV filler emitted in the meantime.
            pe_clock = 0.0
            eng_clock = {"act": 0.0, "dve": 0.0, "gps": 0.0}
            SEMC = 240.0  # ~100ns handoff latency in PE cycles
            chunk_end = []
            pvq = []  # list of (unit_idx, pe_cost, closure), FIFO
            qi = 0

            def drain_pv(upto_unit=None, clock_limit=None, nsteps=None):
                nonlocal qi, pe_clock
                done = 0
                while qi < len(pvq):
                    uidx, cost, fn = pvq[qi]
                    if upto_unit is not None and uidx > upto_unit:
                        break
                    if clock_limit is not None and pe_clock >= clock_limit:
                        break
                    if nsteps is not None and done >= nsteps:
                        break
                    fn()
                    pe_clock += cost
                    qi += 1
                    done += 1

            head_bufs = [None] * HPC
            head_bufs[0] = load_head(0)
            uidx = 0
            for h in range(HPC):
                if h + 1 < HPC:
                    head_bufs[h + 1] = load_head(h + 1)
                if h == HPC - 1:
                    gs = [int(x) for x in _os.environ.get(
                        "LAST_ORDER", "2,3,1,0").split(",")]
                elif h == 0:
                    gs = H0_ORDER
                else:
                    gs = G_ORDER
                for g in gs:
                    # pt pool has PT_BUFS slots: before unit uidx's first exp
                    # can run, unit uidx-PT_BUFS's PV (the slot's previous
                    # holder's reader) must be fully emitted on PE's stream.
                    drain_pv(upto_unit=uidx - PT_BUFS)
                    last = uidx == NG * HPC - 1
                    bufs_u = head_bufs[h]
                    if h == 0 and g == 0:
                        bufs_u = (p0_qt, p0_kt) + tuple(bufs_u[2:])
                    u = make_unit(h, g, bufs_u, split_all=last)
                    if last:
                        # endgame: per-j chunks; leftover PV of previous units
                        # drains under the first exp; each own PV chain goes
                        # right after the per-j exp it depends on
                        own = list(pv_steps(u, split_store=True))
                        oi = 0
                        for ci, (mms, mmcost, ex, eng, ecost) in enumerate(
                            s_chunks(u)
                        ):
                            if ci == 0:
                                drain_pv()
                            for mm in mms:
                                mm()
                            ex()
                            for _ in range(ci + 2):  # ~c+1 matmuls + fin
                                if oi < len(own):
                                    own[oi][1]()
                                    oi += 1
                        while oi < len(own):
                            own[oi][1]()
                            oi += 1
                        continue
                    for mms, mmcost, ex, eng, ecost in s_chunks(u):
                        # give PE filler work until this chunk's exp engine
                        # and its PSUM slot are about to be available
                        target = eng_clock[eng]
                        if len(chunk_end) >= ST_BUFS:
                            target = max(target, chunk_end[-ST_BUFS])
                        drain_pv(clock_limit=target - mmcost)
                        for mm in mms:
                            mm()
                        pe_clock += mmcost
                        if eng != "act" and EX_DELAY:
                            drain_pv(nsteps=EX_DELAY)
                        ex()
                        e_end = max(eng_clock[eng], pe_clock + SEMC) + ecost
                        eng_clock[eng] = e_end
                        chunk_end.append(e_end)
                    pvq.extend(
                        (uidx, cost, fn) for cost, fn in pv_steps(u)
                    )
                    uidx += 1
            drain_pv()

    nc.compile()
    return nc


_F8 = ml_dtypes.float8_e4m3


def _e4m3(x):
    return np.clip(x, -240.0, 240.0).astype(_F8)


def _prep_core(q, k, v):
    """q,k,v: [HPC, S, D] f32 for one core -> device input dict."""
    qtf = np.ascontiguousarray(q.transpose(0, 2, 1))  # [HPC, d, S] f32
    ktf = np.ascontiguousarray(k.transpose(0, 2, 1))
    qt = qtf.astype(_BF16)
    kt = ktf.astype(_BF16)
    va = np.empty((HPC, S, VW), dtype=np.float32)
    va[:, :, :D] = v
    va[:, :, D] = 1.0
    # [HPC, S, VW] -> [HPC, 128, NJ*VW]  with [p, j*VW+c] = va[j*128+p, c]
    va = np.ascontiguousarray(
        va.reshape(HPC, NJ, QB, VW).transpose(0, 2, 1, 3)
    ).reshape(HPC, QB, NJ * VW).astype(_BF16)
    m = {"qt": qt, "kt": kt, "va": va}
    fp8_js, fp8_gs = _CACHE.get("fp8", ([], []))
    if fp8_js:
        # K at double-fp8 (hi + lo/16), Q one-sided e4m3 (+ /16 copy for the
        # DoubleRow second slot)
        kcols = np.concatenate(
            [ktf[:, :, j * QB:(j + 1) * QB] for j in fp8_js], axis=2
        )
        k_hi = _e4m3(kcols)
        k_lo = _e4m3((kcols - k_hi.astype(np.float32)) * 16.0)
        m["khl"] = np.stack([k_hi, k_lo], axis=2)  # [HPC, 128, 2, NJ8*QB]
        qcols = np.concatenate(
            [qtf[:, :, g * GW:(g + 1) * GW] for g in fp8_gs], axis=2
        )
        q8 = _e4m3(qcols)
        q8s = (q8.astype(np.float32) / 16.0).astype(_F8)
        m["q8"] = np.stack([q8, q8s], axis=2)      # [HPC, 128, 2, NG8*GW]
    return m


def _run(query, key, value, trace=False):
    from concourse import bass_utils

    if "nc" not in _CACHE:
        _CACHE["nc"] = _build()
    nc = _CACHE["nc"]

    q = np.asarray(query, dtype=np.float32).reshape(B * H, S, D)
    k = np.asarray(key, dtype=np.float32).reshape(B * H, S, D)
    v = np.asarray(value, dtype=np.float32).reshape(B * H, S, D)
    if _CACHE["mask_mm"]:
        negi = (-60000.0 * np.eye(128, dtype=np.float32)).astype(_BF16)
    else:
        negi = np.triu(np.ones((128, 128), dtype=np.float32)).astype(_BF16)
    low1 = np.tril(np.ones((128, 128), dtype=np.float32), -1).astype(_BF16)

    in_maps = []
    for c in range(N_CORES):
        sl = slice(c * HPC, (c + 1) * HPC)
        m = _prep_core(q[sl], k[sl], v[sl])
        # packed startup tile: [negI | lower1 | kt0 0:512 | qt0 0:512]
        m["p0"] = np.concatenate(
            [negi, low1, m["kt"][0][:, :512], m["qt"][0][:, :512]], axis=1
        )
        in_maps.append(m)

    res = bass_utils.run_bass_kernel_spmd(
        nc, in_maps, core_ids=list(range(N_CORES)), trace=trace
    )
    outs = [res.results[c]["out"] for c in range(N_CORES)]
    full = np.concatenate(outs, axis=0).astype(np.float32)
    # device layout is partition-major: out[h, p, j*OW + d] = row q = j*128+p
    OW = full.shape[-1] // NJ
    full = full.reshape(B * H, QB, NJ, OW).transpose(0, 2, 1, 3)
    full = np.ascontiguousarray(full).reshape(B * H, S, OW)
    if _CACHE.get("norm_host"):
        full = full[..., :D] / full[..., D:D + 1]
    full = full.reshape(B, H, S, D)
    return full, res


def kernel(query, key, value, mask=None):
    """Full inputs in, full output out. `mask` is the causal mask from
    setup_inputs (strictly-upper-triangular True = disallowed); causality is
    implemented structurally so the tensor itself is not consumed."""
    out, _ = _run(query, key, value, trace=False)
    return out



# revision 20
# speedup vs baseline: 1.3040x; 1.0072x over previous
"""Causal multi-head attention on 8 TRN2 NeuronCores.

Problem: B=4, H=16, S=2048, D=128 fp32 causal attention.
Sharding: batch*heads (64) split 8-per-core across the 8 cores; each core
computes its heads fully independently (no collectives).

Per-core kernel strategy (f32 accumulation):
  - scores computed TRANSPOSED: S^T[k,q] = K_j @ Q^T per (k-block j of 128,
    q-group g of 512), causal blocks only, into 2-bank PSUM chunks (<=1024)
    with a 3-deep chunk pipeline
  - ALL non-diagonal score blocks of groups g2/g3 run as fp8e4 DoubleRow
    matmuls at 2x PE rate (cost-model 0.5 cycles/row): the stationary K side
    carries hi + lo/16 double-fp8 (~8 mantissa bits), the moving Q side is
    one-sided e4m3 with a /16 second slot. Host pre-quantizes Q/K; measured
    output rel err ~1.0% vs the 2e-2 gate. fp8 on g0/g1/diagonal blocks is
    deliberately avoided: early rows have few keys and large output norms,
    so they dominate the error weighting.
  - exp mostly on ScalarE (PSUM -> SBUF bf16, 1/sqrt(D) folded into the
    activation scale); four late chunks per head offloaded to VectorE via an
    exp2 bit-trick (one tensor_scalar: bf16 bits = rint(x*log2e*128 +
    127*128 - 7.5), f32->int16 convert aliased over the bf16 tile)
  - diagonal 128x128 blocks masked with a constant triangular tile on DVE
  - PV: out[q,:] = P^T_slice.T @ [V_j | ones]; the appended ones column
    yields the softmax denominator in the same accumulation. Output lands
    directly in [q, d] layout.
  - normalize with VectorE reciprocal + per-partition tensor_scalar mult
  - packed single-DMA startup tile (mask consts + first K/Q block); per-head
    unit order g0,g3,g1,g2 balances exp-column supply against PV drains;
    per-engine pacing clocks keep the PE fed with PV filler work
"""

import sys

import numpy as np
import ml_dtypes

for _p in ("/opt/trn_rl_repo", "/root/.axon_site/_ro/trn_rl_repo"):
    try:
        import concourse  # noqa: F401
        break
    except ImportError:
        if _p not in sys.path:
            sys.path.append(_p)

B, H, S, D = 4, 16, 2048, 128
N_CORES = 8
HPC = (B * H) // N_CORES  # heads per core = 8
QB = 128                  # q/k block
GW = 512                  # q group width
NG = S // GW              # 4 groups per head
NJ = S // QB              # 16 k blocks
VW = D + 1                # V augmented with ones column = 129
SCALE = 1.0 / float(np.sqrt(D))

_BF16 = ml_dtypes.bfloat16

_CACHE = {}


def _build():
    import concourse.bass as bass  # noqa: F401
    import concourse.mybir as mybir
    from concourse import bacc
    from concourse.tile import TileContext

    f32 = mybir.dt.float32
    bf16 = mybir.dt.bfloat16
    i16 = mybir.dt.int16
    EXP = mybir.ActivationFunctionType.Exp
    # chunks routed to the bit-trick exp on VectorE / GpSimd: (g, chunk_idx).
    # Late chunks only — their PV consumption comes last in each q-chain, so
    # the slower engines' latency hides behind ScalarE's pipeline.
    import os as _os
    _dve = _os.environ.get("DVE_SET",
                           "(3,0);(3,1);(3,2);(3,3);(3,4);(3,5)")
    _gps = _os.environ.get("GPS_SET", "")

    def _parse(s):
        out = set()
        for part in s.split(";"):
            part = part.strip().strip("()")
            if part:
                a, b = part.split(",")
                out.add((int(a), int(b)))
        return out

    DVE_CHUNKS = _parse(_dve)
    GPS_CHUNKS = _parse(_gps)
    # per-chunk column split: DVE takes the LAST `SPLIT_COLS` columns of each
    # non-diag ACT chunk (short DVE ops so diag masks aren't queued behind
    # long ones), ScalarE the rest
    SPLIT_COLS = int(_os.environ.get("SPLIT_COLS", "54"))
    NORM_GPS = _os.environ.get("NORM_GPS", "0") == "1"
    # PV steps drained between an offloaded chunk's matmuls and its exp
    # emission: their recip/norm ops enter the DVE FIFO before the exp, so
    # the exp's wait-for-matmuls doesn't head-of-line-block them
    EX_DELAY = int(_os.environ.get("EX_DELAY", "0"))
    # NORM_HOST: the device ships the unnormalized numerator plus the
    # denominator column (VW wide), and the host performs the final divide.
    # (GPSIMD cannot read PSUM, so the acc->SBUF stage stays on DVE either
    # way; host-normalize still drops the reciprocal+multiply.)
    NORM_HOST = _os.environ.get("NORM_HOST", "1") == "1"
    # MASK_GPS: diag masks (SBUF-only tensor_mul) run on the idle GpSimd
    MASK_GPS = _os.environ.get("MASK_GPS", "1") == "1"
    # STAGE_ACT: the acc->SBUF stage runs on ScalarE (activation Copy)
    # instead of DVE; STAGE_BF16: stage+store in bf16 (halves store traffic)
    STAGE_ACT = _os.environ.get("STAGE_ACT", "0") == "1"
    STAGE_BF16 = _os.environ.get("STAGE_BF16", "1") == "1"
    # per-head unit order (steady-state heads): balances exp-column supply
    # against PV-drain bursts at head boundaries
    G_ORDER = [int(x) for x in
               _os.environ.get("G_ORDER", "0,3,1,2").split(",")]
    # head 0 ramps with incremental data needs (g1 needs far less than g3),
    # so its unit order favors load streaming over engine balance
    H0_ORDER = [int(x) for x in
                _os.environ.get("H0_ORDER", "0,1,2,3").split(",")]
    ACT_COST_SCALE = float(_os.environ.get("ACT_COST_SCALE", "0.85"))
    ST_BUFS = int(_os.environ.get("ST_BUFS", "3"))
    ACC_BUFS = int(_os.environ.get("ACC_BUFS", "2"))
    PT_BUFS = int(_os.environ.get("PT_BUFS", "3"))
    MASK_MM = _os.environ.get("MASK_MM", "1") == "1"
    # non-diag chunks whose score matmuls run as fp8e4 DoubleRow (2x PE rate;
    # K carried at hi+lo/16 double-fp8 precision, Q one-sided e4m3).
    # FP8_DIAG_GS: groups whose DIAGONAL blocks also run fp8 (g0's diagonal
    # carries the highest softmax output weight — early rows have few keys
    # and large output norms — so it stays bf16).
    FP8_CHUNKS = _parse(_os.environ.get(
        "FP8_SET", "(1,0);(1,1);"
                   "(2,0);(2,1);(2,2);(2,3);"
                   "(3,0);(3,1);(3,2);(3,3);(3,4);(3,5)"))
    FP8_DIAG_GS = {int(x) for x in
                   _os.environ.get("FP8_DIAG_GS", "1,2,3").split(",")
                   if x.strip()}
    FP8_JS = sorted(
        {j for (g, ci) in FP8_CHUNKS for j in (2 * ci, 2 * ci + 1)
         if j < 4 * g}
        | {4 * g + i for g in FP8_DIAG_GS for i in range(4)}
    )
    FP8_GS = sorted({g for (g, ci) in FP8_CHUNKS} | FP8_DIAG_GS)
    NJ8 = len(FP8_JS)
    J8_COL = {j: i * QB for i, j in enumerate(FP8_JS)}   # khl col offset
    G8_COL = {g: i * GW for i, g in enumerate(FP8_GS)}   # q8 col offset
    # bf16 residual needs: which qt group windows / kt j-blocks still load
    BF16_GS = sorted(
        {g for g in range(NG) for ci in range(2 * g)
         if (g, ci) not in FP8_CHUNKS}
        | {g for g in range(NG) if g not in FP8_DIAG_GS}
    )
    BF16_JS = sorted(
        {j for g in range(NG) for ci in range(2 * g)
         if (g, ci) not in FP8_CHUNKS for j in (2 * ci, 2 * ci + 1)}
        | {4 * g + i for g in range(NG) if g not in FP8_DIAG_GS
           for i in range(4)}
    )
    assert 0 not in FP8_DIAG_GS, "g0 diag must stay bf16 (p0 startup path)"
    _CACHE["mask_mm"] = MASK_MM
    KT_COL = {j: i * QB for i, j in enumerate(BF16_JS)}
    QT_COL = {g: i * GW for i, g in enumerate(BF16_GS)}
    C1E = float(np.log2(np.e) * 128.0 * SCALE)
    C2E = 127.0 * 128.0 - 7.5

    nc = bacc.Bacc("TRN2", target_bir_lowering=False, num_devices=N_CORES)

    qt_d = nc.dram_tensor("qt", [HPC, 128, S], bf16, kind="ExternalInput").ap()
    kt_d = nc.dram_tensor("kt", [HPC, 128, S], bf16, kind="ExternalInput").ap()
    va_d = nc.dram_tensor("va", [HPC, 128, NJ * VW], bf16, kind="ExternalInput").ap()
    # packed startup tile: [negI | lower1 | kt0 0:512 | qt0 0:512] —
    # mask-matmul constants plus everything head-0's g0 diag chunk needs,
    # fetched in a single DMA (per-DMA fixed costs dominate the startup
    # critical path)
    p0_d = nc.dram_tensor("p0", [128, 2 * QB + 2 * GW], bf16,
                          kind="ExternalInput").ap()
    f8e4 = mybir.dt.float8e4
    DRPM = mybir.MatmulPerfMode.DoubleRow
    if NJ8:
        khl_d = nc.dram_tensor("khl", [HPC, 128, 2, NJ8 * QB], f8e4,
                               kind="ExternalInput").ap()
        q8_d = nc.dram_tensor("q8", [HPC, 128, 2, len(FP8_GS) * GW], f8e4,
                              kind="ExternalInput").ap()
    _CACHE["fp8"] = (FP8_JS, FP8_GS)
    _CACHE["norm_host"] = NORM_HOST
    OW = VW if NORM_HOST else D   # output row width
    out_dt = bf16 if STAGE_BF16 else f32
    _CACHE["out_bf16"] = STAGE_BF16
    # partition-major out layout: out[h, p, j*OW + d] holds row q = j*128 + p.
    # Stores become fully contiguous per partition (4*OW*2B = 1032B runs at
    # full DMA bus rate vs 258B rows at half rate); host un-shuffles.
    out_d = nc.dram_tensor("out", [HPC, 128, NJ * OW], out_dt,
                           kind="ExternalOutput").ap()

    with TileContext(nc) as tc:
        with (
            tc.tile_pool(name="consts", bufs=1) as consts,
            tc.tile_pool(name="io", bufs=3) as io,
            tc.tile_pool(name="pt", bufs=PT_BUFS) as ptp,
            tc.tile_pool(name="ob", bufs=4) as obp,
            tc.tile_pool(name="rr", bufs=4) as rrp,
            tc.tile_pool(name="st", bufs=ST_BUFS, space="PSUM") as stp,
            tc.tile_pool(name="acc", bufs=ACC_BUFS, space="PSUM") as accp,
        ):
            # packed startup tile [negI | lower1 | kt0 0:512 | qt0 0:512]:
            # one DMA with one completion-sem covers everything unit (h0, g0)
            # consumes. negI/lower1 implement causal masking INSIDE the score
            # accumulation: st[k,q] += -60000*[k>q] via one extra 128-col
            # matmul per diag block, so exp yields exact zeros above the
            # diagonal and no post-exp mask op exists on any engine.
            p0_sb = consts.tile([128, 2 * QB + 2 * GW], bf16, name="p0_sb")
            nc.sync.dma_start(out=p0_sb[:, :], in_=p0_d[:, :])
            negi_sb = p0_sb[:, 0:QB]
            low1_sb = p0_sb[:, QB:2 * QB]
            p0_kt = p0_sb[:, 2 * QB:2 * QB + GW]
            p0_qt = p0_sb[:, 2 * QB + GW:2 * QB + 2 * GW]
            # HAM warm-up: PE idles ~2us at start waiting for the first DMA
            # anyway; dummy matmuls on memset SBUF keep the PE activity
            # monitor busy so real work starts fast (real-HW; near-neutral in
            # sim). PSUM target is overwritten by start=True.
            warm_in = consts.tile([128, VW], bf16, name="warm_in")
            nc.vector.memset(warm_in[:, :], 0.0)
            # hoist the ACT table load (real-HW ~2.7us incl drain) into the
            # startup DMA window via a dummy 1-col activation
            warm_ex = consts.tile([128, 1], bf16, name="warm_ex")
            nc.scalar.activation(warm_ex[:, :], warm_in[:, 0:1], EXP,
                                 scale=SCALE)
            warm_acc = accp.tile([128, VW], f32, tag="acc", name="warm_acc")
            for _ in range(16):
                nc.tensor.matmul(
                    warm_acc[:, :], lhsT=warm_in[:, 0:QB],
                    rhs=warm_in[:, 0:VW], start=True, stop=True,
                )

            def load_head(h):
                """bf16 qt/kt tiles hold only the group-windows / j-blocks
                still computed in bf16 (KT_COL/QT_COL give their offsets);
                fp8 khl/q8 carry everything else. All loads issue on SP
                (HWDGE) so the Pool engine stays free for the diag masks."""
                nq = max(len(BF16_GS), 1)
                nk = max(len(BF16_JS), 1)
                qt_sb = io.tile([128, nq * GW], bf16, tag="qt", name=f"qt{h}")
                kt_sb = io.tile([128, nk * QB], bf16, tag="kt", name=f"kt{h}")
                va_sb = io.tile([128, NJ * VW], bf16, tag="va", name=f"va{h}")
                if NJ8:
                    khl_sb = io.tile([128, 2, NJ8 * QB], f8e4, tag="khl",
                                     name=f"khl{h}")
                    q8_sb = io.tile([128, 2, len(FP8_GS) * GW], f8e4,
                                    tag="q8", name=f"q8{h}")
                else:
                    khl_sb = q8_sb = None
                m = (NJ * VW) // 2
                skip_p0 = h == 0
                # p0 fully covers head 0's bf16 needs when the bf16 residue
                # is exactly g0's diagonal (all-fp8 nd + fp8 diag g1..g3)
                p0_covers = (skip_p0 and BF16_JS == [0, 1, 2, 3]
                             and BF16_GS == [0])

                def kt_runs():
                    runs = []
                    for j in BF16_JS:
                        if runs and runs[-1][1] == j:
                            runs[-1][1] = j + 1
                        else:
                            runs.append([j, j + 1])
                    return runs

                def kt_piece(j0, j1):
                    if j0 >= j1:
                        return
                    nc.sync.dma_start(
                        out=kt_sb[:, KT_COL[j0]:KT_COL[j0] + (j1 - j0) * QB],
                        in_=kt_d[h, :, j0 * QB:j1 * QB])

                def qt_piece(g):
                    nc.sync.dma_start(
                        out=qt_sb[:, QT_COL[g]:QT_COL[g] + GW],
                        in_=qt_d[h, :, g * GW:(g + 1) * GW])

                def q8_piece(g):
                    c = G8_COL[g]
                    nc.sync.dma_start(out=q8_sb[:, :, c:c + GW],
                                      in_=q8_d[h, :, :, c:c + GW])

                if not p0_covers:
                    for j0, j1 in kt_runs():
                        kt_piece(j0, j1)
                    for g in BF16_GS:
                        if skip_p0 and g == 0:
                            continue
                        qt_piece(g)
                # stream in first-use order of the head's unit order:
                # va first half feeds g0's PV early; q8 pieces follow the
                # group order; khl halves bracket them
                order = H0_ORDER if h == 0 else G_ORDER
                # head 0 ramps with no compute to hide loads under: split the
                # stream across HWDGE (SP) and SWDGE (Pool) so both DGE paths
                # generate descriptors in parallel
                gps = nc.gpsimd if h == 0 else nc.sync
                nc.sync.dma_start(out=va_sb[:, 0:m], in_=va_d[h, :, 0:m])
                if NJ8:
                    half = (NJ8 * QB) // 2
                    gps.dma_start(out=khl_sb[:, :, 0:half],
                                  in_=khl_d[h, :, :, 0:half])
                    gs8 = [g for g in order if g in FP8_GS]
                    if gs8:
                        q8_piece(gs8[0])
                    gps.dma_start(out=khl_sb[:, :, half:],
                                  in_=khl_d[h, :, :, half:])
                    for g in gs8[1:2]:
                        q8_piece(g)
                    nc.sync.dma_start(out=va_sb[:, m:], in_=va_d[h, :, m:])
                    for g in gs8[2:]:
                        q8_piece(g)
                else:
                    nc.sync.dma_start(out=va_sb[:, m:], in_=va_d[h, :, m:])
                return qt_sb, kt_sb, va_sb, khl_sb, q8_sb

            def s_chunks(u):
                """Per chunk of unit u: (mm_closures_with_cost, exp_closure,
                act_cost). S^T matmuls land in bank-aligned PSUM chunks
                (<=3 banks), one exp per chunk, diag masks after the exp."""
                h, g, bufs, pt_sb, offs, chunks, split_all = u
                qt_sb, kt_sb = bufs[0], bufs[1]
                khl_sb, q8_sb = bufs[3], bufs[4]
                qhi = GW * (g + 1)
                for ci, (col0, entries, cw) in enumerate(chunks):
                    st = stp.tile([128, 1024], f32, tag="st",
                                  name=f"st{h}g{g}c{ci}")
                    mms = []
                    mmcost = 0
                    fp8c = (g, ci) in FP8_CHUNKS and all(
                        j < 4 * g for (j, _q, _o, _w) in entries
                    )
                    for (j, qlo, off, w) in entries:
                        if fp8c or (j >= 4 * g and g in FP8_DIAG_GS):
                            diag_mask = j >= 4 * g and MASK_MM

                            def mm(j=j, qlo=qlo, off=off, w=w, st=st, g=g,
                                   diag_mask=diag_mask):
                                c8 = G8_COL[g] + (qlo - GW * g)
                                nc.tensor.matmul(
                                    st[:, off:off + w],
                                    lhsT=khl_sb[
                                        :, :, J8_COL[j]:J8_COL[j] + QB
                                    ],
                                    rhs=q8_sb[:, :, c8:c8 + w],
                                    start=True, stop=not diag_mask,
                                    perf_mode=DRPM,
                                )
                                if diag_mask:
                                    # causal mask inside the accumulation:
                                    # st[k,q] += -60000*[k>q] on the leading
                                    # 128-col diagonal square of the entry
                                    nc.tensor.matmul(
                                        st[:, off:off + QB],
                                        lhsT=negi_sb[:, :],
                                        rhs=low1_sb[:, :],
                                        start=False, stop=True,
                                        skip_group_check=True,
                                    )
                            mms.append(mm)
                            mmcost += w // 2 + 8
                            if diag_mask:
                                mmcost += QB + 8
                        elif j >= 4 * g and MASK_MM:
                            # diag block: accumulate -60000 above the diagonal
                            # in the same PSUM group (masking via the PE)
                            def mm(j=j, qlo=qlo, off=off, w=w, st=st, g=g):
                                cq = QT_COL[g] + (qlo - GW * g)
                                nc.tensor.matmul(
                                    st[:, off:off + w],
                                    lhsT=kt_sb[:, KT_COL[j]:KT_COL[j] + QB],
                                    rhs=qt_sb[:, cq:cq + w],
                                    start=True, stop=False,
                                )
                                nc.tensor.matmul(
                                    st[:, off:off + QB],
                                    lhsT=negi_sb[:, :],
                                    rhs=low1_sb[:, :],
                                    start=False, stop=True,
                                    skip_group_check=True,
                                )
                            mms.append(mm)
                            mmcost += w + QB + 16
                        else:
                            def mm(j=j, qlo=qlo, off=off, w=w, st=st, g=g):
                                cq = QT_COL[g] + (qlo - GW * g)
                                nc.tensor.matmul(
                                    st[:, off:off + w],
                                    lhsT=kt_sb[:, KT_COL[j]:KT_COL[j] + QB],
                                    rhs=qt_sb[:, cq:cq + w],
                                    start=True, stop=True,
                                )
                            mms.append(mm)
                            mmcost += w + 8

                    # offload a slice of the exp work to the idle VectorE /
                    # GpSimd engines via the exp2 bit-trick: bf16 bits =
                    # rint(x*log2e*128 + C2E) (one tensor_scalar, f32->int16
                    # convert aliased over the bf16 tile). ~1.3% element
                    # error, bias-centered so the softmax output error stays
                    # small. (the split-all final unit keeps everything on
                    # ScalarE)
                    final = split_all
                    nd_chunk = all(j < 4 * g for (j, _q, _o, _w) in entries)
                    eng = "act"
                    if not final and (nd_chunk or not MASK_MM):
                        # (with MASK_MM the bit-trick's f32->int16 convert
                        # would wrap on the -60000 masked scores, so diag
                        # chunks are only offloadable on the DVE-mask path)
                        if (g, ci) in DVE_CHUNKS:
                            eng = "dve"
                        elif (g, ci) in GPS_CHUNKS:
                            eng = "gps"

                    sl = SPLIT_COLS if (eng == "act" and nd_chunk and not final
                                        and SPLIT_COLS < cw) else 0

                    def ex(col0=col0, cw=cw, st=st, entries=entries, eng=eng,
                           sl=sl):
                        if eng == "act":
                            nc.scalar.activation(
                                pt_sb[:, col0:col0 + cw - sl],
                                st[:, 0:cw - sl], EXP, scale=SCALE,
                            )
                            if sl:
                                nc.vector.tensor_scalar(
                                    pt_sb[
                                        :, col0 + cw - sl:col0 + cw
                                    ].bitcast(i16),
                                    st[:, cw - sl:cw], C1E, C2E,
                                    mybir.AluOpType.mult, mybir.AluOpType.add,
                                )
                        else:
                            e = nc.vector if eng == "dve" else nc.gpsimd
                            e.tensor_scalar(
                                pt_sb[:, col0:col0 + cw].bitcast(i16),
                                st[:, 0:cw], C1E, C2E,
                                mybir.AluOpType.mult, mybir.AluOpType.add,
                            )
                        if not MASK_MM:
                            # negi_sb slot holds the inclusive upper-tri mask
                            # in this mode (host-selected)
                            me = nc.gpsimd if MASK_GPS else nc.vector
                            for (j, qlo, off, w) in entries:
                                if j >= 4 * g:  # diag: zero where k > q
                                    me.tensor_mul(
                                        pt_sb[:, col0 + off:col0 + off + QB],
                                        pt_sb[:, col0 + off:col0 + off + QB],
                                        negi_sb[:, :],
                                    )
                    if eng == "act":
                        ecost = int(2 * (cw - sl + 222) * ACT_COST_SCALE)
                    elif eng == "dve":
                        ecost = int(2.5 * cw) + 300
                    else:
                        ecost = int(3.33 * cw) + 700
                    yield mms, mmcost, ex, eng, ecost

            def pv_steps(u, split_store=False, store_eng=None):
                """(pe_cost, closure) steps: PV accumulation matmuls +
                stage + store for unit u. acc tiles hold HALF a unit
                (2 q-blocks, exactly 1 PSUM bank) so 3 score chunks + 2 accs
                fit the 8 banks. Staging happens per half (one 2*OW-col op),
                the store once per unit (split_store also stores the first
                half early for the endgame tail)."""
                h, g, bufs, pt_sb, offs, _chunks, _sa = u
                va_sb = bufs[2]
                if store_eng is None:
                    store_eng = nc.sync
                o_grp = obp.tile([128, 4 * OW], out_dt, tag="obg",
                                 name=f"og{h}g{g}")
                acc = None
                for c in range(4):
                    Q = 4 * g + c
                    qlo_c = GW * g + QB * c
                    if c % 2 == 0:
                        acc = accp.tile([128, 2 * VW], f32, tag="acc",
                                        name=f"acc{h}g{g}c{c}")
                    a0 = (c % 2) * VW
                    for j in range(Q + 1):
                        qlo_j, col_j = offs[j]
                        off = col_j + (qlo_c - qlo_j)

                        def step(j=j, Q=Q, off=off, acc=acc, a0=a0):
                            nc.tensor.matmul(
                                acc[:, a0:a0 + VW],
                                lhsT=pt_sb[:, off:off + QB],
                                rhs=va_sb[:, j * VW:(j + 1) * VW],
                                start=(j == 0), stop=(j == Q),
                            )
                        yield 300, step

                    if c % 2 == 0:
                        continue

                    def fin(c=c, acc=acc, o_grp=o_grp):
                        ob0 = (c - 1) * OW
                        if NORM_HOST:
                            # ship numerator+denominator (only ACT/DVE can
                            # read PSUM); host divides. One 2*OW-wide stage
                            # per half-unit.
                            if STAGE_ACT:
                                nc.scalar.activation(
                                    o_grp[:, ob0:ob0 + 2 * OW],
                                    acc[:, :],
                                    mybir.ActivationFunctionType.Copy,
                                    scale=1.0,
                                )
                            else:
                                nc.vector.tensor_copy(
                                    o_grp[:, ob0:ob0 + 2 * OW],
                                    acc[:, :],
                                )
                        else:
                            for cc in (c - 1, c):
                                a0 = (cc % 2) * VW
                                r = rrp.tile([128, 1], f32, tag="r",
                                             name=f"r{h}g{g}c{cc}")
                                nc.vector.reciprocal(
                                    r[:, :], acc[:, a0 + D:a0 + D + 1])
                                nc.vector.tensor_scalar_mul(
                                    o_grp[:, cc * OW:(cc + 1) * OW],
                                    acc[:, a0:a0 + D], r[:, :],
                                )
                        if split_store and c == 1:
                            store_eng.dma_start(
                                out=out_d[h, :,
                                          4 * g * OW:(4 * g + 2) * OW],
                                in_=o_grp[:, 0:2 * OW])
                        elif split_store and c == 3:
                            store_eng.dma_start(
                                out=out_d[h, :,
                                          (4 * g + 2) * OW:(4 * g + 4) * OW],
                                in_=o_grp[:, 2 * OW:4 * OW])
                        elif not split_store and c == 3:
                            store_eng.dma_start(
                                out=out_d[h, :,
                                          4 * g * OW:(4 * g + 4) * OW],
                                in_=o_grp[:, :])
                    yield 0, fin

            def make_unit(h, g, bufs, split_all=False):
                # Chunk layout: non-diag js in twos (512 each, bank aligned),
                # then the diag chunk packed 512+384 | 256+128 into 2.5 banks.
                # chunks: list of (pt_col0, [(j, qlo, off_in_chunk, w)], width)
                chunks = []
                col = 0
                nd = 4 * g  # non-diagonal k-blocks
                for i0 in range(0, nd, 2):
                    entries = [
                        (j, GW * g, (j - i0) * GW, GW)
                        for j in range(i0, min(i0 + 2, nd))
                    ]
                    cw = len(entries) * GW
                    chunks.append((col, entries, cw))
                    col += cw
                d0 = 4 * g
                if split_all:
                    # per-j chunks (used for the final unit so its PV can
                    # begin before the whole diagonal chunk is exp'd)
                    for i, w in enumerate((512, 384, 256, 128)):
                        chunks.append(
                            (col, [(d0 + i, QB * (d0 + i), 0, w)], w)
                        )
                        col += w
                else:
                    chunks.append((col, [
                        (d0, QB * d0, 0, 512),
                        (d0 + 1, QB * (d0 + 1), 512, 384),
                    ], 896))
                    col += 896
                    chunks.append((col, [
                        (d0 + 2, QB * (d0 + 2), 0, 256),
                        (d0 + 3, QB * (d0 + 3), 256, 128),
                    ], 384))
                    col += 384
                offs = {}
                for col0, entries, _ in chunks:
                    for (j, qlo, off, _w) in entries:
                        offs[j] = (qlo, col0 + off)
                pt_sb = ptp.tile(
                    [128, 12 * GW + 1280], bf16, tag="pt", name=f"pt{h}g{g}"
                )
                return (h, g, bufs, pt_sb, offs, chunks, split_all)

            # Global clock-based pacing: emit exp chunks on each exp-engine's
            # schedule, fill PE's spare time from a queue of pending PV work.
            # Clocks in PE cycles @2.4GHz; ACT cycles count double, DVE 2.5x,
            # GPS 3.33x. chunk_end tracks modeled exp completions so chunk
            # k's matmuls are delayed until chunk k-3's PSUM slot frees
            # (3-slot st pool) with PV filler emitted in the meantime.
            pe_clock = 0.0
            eng_clock = {"act": 0.0, "dve": 0.0, "gps": 0.0}
            SEMC = 240.0  # ~100ns handoff latency in PE cycles
            chunk_end = []
            pvq = []  # list of (unit_idx, pe_cost, closure), FIFO
            qi = 0

            def drain_pv(upto_unit=None, clock_limit=None, nsteps=None):
                nonlocal qi, pe_clock
                done = 0
                while qi < len(pvq):
                    uidx, cost, fn = pvq[qi]
                    if upto_unit is not None and uidx > upto_unit:
                        break
                    if clock_limit is not None and pe_clock >= clock_limit:
                        break
                    if nsteps is not None and done >= nsteps:
                        break
                    fn()
                    pe_clock += cost
                    qi += 1
                    done += 1

            head_bufs = [None] * HPC
            head_bufs[0] = load_head(0)
            uidx = 0
            for h in range(HPC):
                if h + 1 < HPC:
                    head_bufs[h + 1] = load_head(h + 1)
                if h == HPC - 1:
                    gs = [int(x) for x in _os.environ.get(
                        "LAST_ORDER", "2,3,1,0").split(",")]
                elif h == 0:
                    gs = H0_ORDER
                else:
                    gs = G_ORDER
                for g in gs:
                    # pt pool has PT_BUFS slots: before unit uidx's first exp
                    # can run, unit uidx-PT_BUFS's PV (the slot's previous
                    # holder's reader) must be fully emitted on PE's stream.
                    drain_pv(upto_unit=uidx - PT_BUFS)
                    last = uidx == NG * HPC - 1
                    bufs_u = head_bufs[h]
                    if h == 0 and g == 0:
                        bufs_u = (p0_qt, p0_kt) + tuple(bufs_u[2:])
                    u = make_unit(h, g, bufs_u, split_all=last)
                    if last:
                        # endgame: per-j chunks; leftover PV of previous units
                        # drains under the first exp; each own PV chain goes
                        # right after the per-j exp it depends on
                        own = list(pv_steps(u, split_store=True))
                        oi = 0
                        for ci, (mms, mmcost, ex, eng, ecost) in enumerate(
                            s_chunks(u)
                        ):
                            if ci == 0:
                                drain_pv()
                            for mm in mms:
                                mm()
                            ex()
                            for _ in range(ci + 2):  # ~c+1 matmuls + fin
                                if oi < len(own):
                                    own[oi][1]()
                                    oi += 1
                        while oi < len(own):
                            own[oi][1]()
                            oi += 1
                        continue
                    for mms, mmcost, ex, eng, ecost in s_chunks(u):
                        # give PE filler work until this chunk's exp engine
                        # and its PSUM slot are about to be available
                        target = eng_clock[eng]
                        if len(chunk_end) >= ST_BUFS:
                            target = max(target, chunk_end[-ST_BUFS])
                        drain_pv(clock_limit=target - mmcost)
                        for mm in mms:
                            mm()
                        pe_clock += mmcost
                        if eng != "act" and EX_DELAY:
                            drain_pv(nsteps=EX_DELAY)
                        ex()
                        e_end = max(eng_clock[eng], pe_clock + SEMC) + ecost
                        eng_clock[eng] = e_end
                        chunk_end.append(e_end)
                    pvq.extend(
                        (uidx, cost, fn) for cost, fn in pv_steps(u)
                    )
                    uidx += 1
            drain_pv()

    nc.compile()
    return nc


_F8 = ml_dtypes.float8_e4m3


def _e4m3(x):
    return np.clip(x, -240.0, 240.0).astype(_F8)


def _prep_core(q, k, v):
    """q,k,v: [HPC, S, D] f32 for one core -> device input dict."""
    qtf = np.ascontiguousarray(q.transpose(0, 2, 1))  # [HPC, d, S] f32
    ktf = np.ascontiguousarray(k.transpose(0, 2, 1))
    qt = qtf.astype(_BF16)
    kt = ktf.astype(_BF16)
    va = np.empty((HPC, S, VW), dtype=np.float32)
    va[:, :, :D] = v
    va[:, :, D] = 1.0
    # [HPC, S, VW] -> [HPC, 128, NJ*VW]  with [p, j*VW+c] = va[j*128+p, c]
    va = np.ascontiguousarray(
        va.reshape(HPC, NJ, QB, VW).transpose(0, 2, 1, 3)
    ).reshape(HPC, QB, NJ * VW).astype(_BF16)
    m = {"qt": qt, "kt": kt, "va": va}
    fp8_js, fp8_gs = _CACHE.get("fp8", ([], []))
    if fp8_js:
        # K at double-fp8 (hi + lo/16), Q one-sided e4m3 (+ /16 copy for the
        # DoubleRow second slot)
        kcols = np.concatenate(
            [ktf[:, :, j * QB:(j + 1) * QB] for j in fp8_js], axis=2
        )
        k_hi = _e4m3(kcols)
        k_lo = _e4m3((kcols - k_hi.astype(np.float32)) * 16.0)
        m["khl"] = np.stack([k_hi, k_lo], axis=2)  # [HPC, 128, 2, NJ8*QB]
        qcols = np.concatenate(
            [qtf[:, :, g * GW:(g + 1) * GW] for g in fp8_gs], axis=2
        )
        q8 = _e4m3(qcols)
        q8s = (q8.astype(np.float32) / 16.0).astype(_F8)
        m["q8"] = np.stack([q8, q8s], axis=2)      # [HPC, 128, 2, NG8*GW]
    return m


def _run(query, key, value, trace=False):
    from concourse import bass_utils

    if "nc" not in _CACHE:
        _CACHE["nc"] = _build()
    nc = _CACHE["nc"]

    q = np.asarray(query, dtype=np.float32).reshape(B * H, S, D)
    k = np.asarray(key, dtype=np.float32).reshape(B * H, S, D)
    v = np.asarray(value, dtype=np.float32).reshape(B * H, S, D)
    if _CACHE["mask_mm"]:
        negi = (-60000.0 * np.eye(128, dtype=np.float32)).astype(_BF16)
    else:
        negi = np.triu(np.ones((128, 128), dtype=np.float32)).astype(_BF16)
    low1 = np.tril(np.ones((128, 128), dtype=np.float32), -1).astype(_BF16)

    in_maps = []
    for c in range(N_CORES):
        sl = slice(c * HPC, (c + 1) * HPC)
        m = _prep_core(q[sl], k[sl], v[sl])
        # packed startup tile: [negI | lower1 | kt0 0:512 | qt0 0:512]
        m["p0"] = np.concatenate(
            [negi, low1, m["kt"][0][:, :512], m["qt"][0][:, :512]], axis=1
        )
        in_maps.append(m)

    res = bass_utils.run_bass_kernel_spmd(
        nc, in_maps, core_ids=list(range(N_CORES)), trace=trace
    )
    outs = [res.results[c]["out"] for c in range(N_CORES)]
    full = np.concatenate(outs, axis=0).astype(np.float32)
    # device layout is partition-major: out[h, p, j*OW + d] = row q = j*128+p
    OW = full.shape[-1] // NJ
    full = full.reshape(B * H, QB, NJ, OW).transpose(0, 2, 1, 3)
    full = np.ascontiguousarray(full).reshape(B * H, S, OW)
    if _CACHE.get("norm_host"):
        full = full[..., :D] / full[..., D:D + 1]
    full = full.reshape(B, H, S, D)
    return full, res


def kernel(query, key, value, mask=None):
    """Full inputs in, full output out. `mask` is the causal mask from
    setup_inputs (strictly-upper-triangular True = disallowed); causality is
    implemented structurally so the tensor itself is not consumed."""
    out, _ = _run(query, key, value, trace=False)
    return out

